# revision 27
# baseline (speedup 1.0000x reference)
"""Trainium2 Bass kernel for DINO loss (nn_DINOLoss_44083544326419).

kernel(**inputs) takes FULL unsharded inputs and returns (total_loss, new_center),
matching the reference nn.Module. Strategy:

- batch dim (B=256) sharded across 8 NeuronCores (32 samples/core)
- per core, heavy math runs in a d-chunked transposed layout
  ([128 d-partitions x row-columns]); full-row reductions become PE matmuls
  against a ones-vector, accumulated in PSUM across all 512 d-chunks:
    * dots[(i,b),(v,b')] = sum_d exp((t-c)/temp - K) * student
    * zs = sum_d exp(10*student)   (student softmax denominator)
    * ztp = sum_d qt               (teacher softmax denominator, per chunk-phase)
  student log-softmax needs no max-subtraction (|10 s|max ~ 55 fits f32 exp);
  teacher softmax uses a fixed offset K=100 (max u ~ 110 -> exp(u-K) <= e^10).
- center update: scatter-add as a one-hot matmul (one-hot columns pre-scaled by
  0.1/max(2*counts,1) on host), summed across cores with an on-device
  ReduceScatter; each core adds 0.9*center for its d-slice and writes that
  slice of new_center.
- tiny per-(i,v,b) scalar assembly (logs, masking, means) happens on host in
  float64 from the per-core reduction outputs (~85 KB/core).
"""

import os
import sys

import numpy as np

KOFF = 100.0  # teacher exp offset
WARMUP_T = 0.04
TEACHER_T = 0.07
WARMUP_EPOCHS = 30
NEPOCHS = 100


def _teacher_temp(epoch: int) -> float:
    sched = np.concatenate(
        (np.linspace(WARMUP_T, TEACHER_T, WARMUP_EPOCHS),
         np.ones(NEPOCHS - WARMUP_EPOCHS) * TEACHER_T))
    return float(sched[int(epoch)])


def _import_concourse():
    try:
        import concourse.bass  # noqa: F401
    except ImportError:
        for p in ("/opt/trn_rl_repo", "/root/.axon_site/_ro/trn_rl_repo"):
            if os.path.isdir(p) and p not in sys.path:
                sys.path.insert(0, p)
    import concourse.bass as bass
    import concourse.bacc as bacc
    import concourse.mybir as mybir
    import concourse.tile as tile
    from concourse import bass_utils
    return bass, bacc, mybir, tile, bass_utils


class Cfg:
    """Problem geometry. Defaults = the real problem; overridable for sim tests."""

    def __init__(self, D=65536, BL=32, NCROPS=10, NCENT=51, NCORES=8, G=16, SG=2,
                 bf16_student=True):
        self.D = D                    # feature dim
        self.BL = BL                  # local batch per core
        self.NCROPS = NCROPS
        self.NCENT = NCENT
        self.NCORES = NCORES
        self.B = BL * NCORES          # global batch
        self.P = 128                  # d-chunk partition size
        assert D % (self.P * NCORES * 2) == 0
        self.NCHUNK = D // self.P     # number of d-chunks
        self.G = G                    # chunks per student group
        assert self.NCHUNK % G == 0
        self.NGRP = self.NCHUNK // G
        self.SG = SG                  # student groups per teacher supergroup
        assert self.NGRP % SG == 0
        self.NSG = self.NGRP // SG
        self.TC = 2 * BL              # teacher cols per chunk
        self.SC = NCROPS * BL         # student cols per chunk
        assert 2 * self.TC <= 128
        self.DSL = D // NCORES        # d-slice width per core (center output)
        self.DSH = self.DSL // 2      # packed half-slice width
        self.SCB = min(512, self.DSH)          # scatter matmul free-dim block
        assert self.DSH % self.SCB == 0
        self.QW = min(4096, self.DSH)          # scatter staging width
        assert self.DSH % self.QW == 0
        self.PSB = min(1024, self.QW)          # scatter psum tile width
        assert self.QW % self.PSB == 0 and self.PSB % self.SCB == 0
        self.CQW = min(2048, self.DSH)         # phase-C tile width
        assert self.DSH % self.CQW == 0
        self.ZW = min(512, self.G * self.TC)   # ztp matmul slice width
        assert (self.G * self.TC) % self.ZW == 0
        self.bf16_student = bf16_student


def build_kernel(cfg: Cfg, temp: float, use_collective=True):
    """Builds, schedules and compiles the SPMD kernel; returns the Bacc module."""
    bass, bacc, mybir, tile, bass_utils = _import_concourse()
    dt = mybir.dt
    f32 = dt.float32
    # matmul-feeding dtype: bf16, or float32r (f32 bits, 1 cyc/row on PE when
    # the moving dim >= 256; walrus requires producers typed f32r end-to-end)
    dt_s = dt.bfloat16 if cfg.bf16_student else dt.float32r
    dt_r = dt.float32r
    AF = mybir.ActivationFunctionType

    def mmview(ap):
        return ap

    nc = bacc.Bacc("TRN2", target_bir_lowering=False, debug=False,
                   num_devices=cfg.NCORES)

    P, G, SG, TC, SC, NCENT, BL = cfg.P, cfg.G, cfg.SG, cfg.TC, cfg.SC, cfg.NCENT, cfg.BL

    s_t = nc.dram_tensor("s_t", [cfg.NGRP, P, G * SC], dt_s, kind="ExternalInput")
    t_t = nc.dram_tensor("t_t", [cfg.NSG, P, SG * G * TC], f32, kind="ExternalInput")
    c_t = nc.dram_tensor("c_t", [cfg.NSG, P, SG * G * BL], f32, kind="ExternalInput")
    t_n = nc.dram_tensor("t_n", [cfg.NCORES, 2 * TC, cfg.DSH], dt_r,
                         kind="ExternalInput")
    oh = nc.dram_tensor("oh", [2 * TC, NCENT], dt_r, kind="ExternalInput")
    c09 = nc.dram_tensor("c09", [2 * NCENT, cfg.DSH], f32, kind="ExternalInput")

    dots_o = nc.dram_tensor("dots_o", [TC, SC], f32, kind="ExternalOutput")
    zs_o = nc.dram_tensor("zs_o", [1, SC], f32, kind="ExternalOutput")
    ztp_o = nc.dram_tensor("ztp_o", [1, G * TC], f32, kind="ExternalOutput")
    if use_collective:
        nco_o = nc.dram_tensor("nco_o", [2 * NCENT, cfg.DSH], f32,
                               kind="ExternalOutput")
    else:
        bcp_o = nc.dram_tensor("bcp_o", [cfg.NCORES * 2 * NCENT, cfg.DSH], f32,
                               kind="ExternalOutput")

    with tile.TileContext(nc) as tc:
        with (
            tc.tile_pool(name="const", bufs=1) as constp,
            tc.tile_pool(name="tn", bufs=2) as tnp,
            tc.tile_pool(name="stag", bufs=2) as stagp,
            tc.tile_pool(name="spool", bufs=2) as spool,
            tc.tile_pool(name="espool", bufs=2) as espool,
            tc.tile_pool(name="tpool", bufs=2) as tpool,
            tc.tile_pool(name="cpool", bufs=2) as cpool,
            tc.tile_pool(name="qtpool", bufs=2) as qtpool,
            tc.tile_pool(name="opool", bufs=1) as opool,
            tc.tile_pool(name="cph", bufs=1) as cph,
            tc.tile_pool(name="psA", bufs=1, space="PSUM") as psA,
            tc.tile_pool(name="psS", bufs=2, space="PSUM") as psS,
            tc.tile_pool(name="dram", bufs=1, space="DRAM") as dram,
        ):
            ones_t = constp.tile([P, 1], dt_s, name="ones_t")
            nc.gpsimd.memset(ones_t[:], 1.0)
            koff_t = constp.tile([P, 1], f32, name="koff_t")
            nc.gpsimd.memset(koff_t[:], -KOFF)
            oh_t = constp.tile([2 * TC, NCENT], dt_r, name="oh_t")
            nc.gpsimd.dma_start(out=oh_t[:], in_=oh[:])

            if use_collective:
                cc_in = dram.tile([cfg.NCORES * 2 * NCENT, cfg.DSH], f32,
                                  name="cc_in")
                cc_out = dram.tile([2 * NCENT, cfg.DSH], f32, name="cc_out")

            # ---------- phase A slice emitter: scatter-add via one-hot matmul ----
            def phase_a_slice(s):
                tn_t = tnp.tile([2 * TC, cfg.DSH], dt_r, name="tn_t")
                nc.gpsimd.dma_start(out=tn_t[:], in_=t_n[s])
                for half in range(2):
                    pr0 = half * TC
                    row0 = s * 2 * NCENT + half * NCENT
                    for q in range(cfg.DSH // cfg.QW):
                        stag = stagp.tile([NCENT, cfg.QW], f32, name="stag")
                        for pb in range(cfg.QW // cfg.PSB):
                            bc_ps = psS.tile([NCENT, cfg.PSB], f32, name="bc_ps")
                            for m in range(cfg.PSB // cfg.SCB):
                                col = q * cfg.QW + pb * cfg.PSB + m * cfg.SCB
                                nc.tensor.matmul(
                                    bc_ps[:, m * cfg.SCB:(m + 1) * cfg.SCB],
                                    mmview(oh_t[pr0:pr0 + TC, :]),
                                    mmview(tn_t[pr0:pr0 + TC, col:col + cfg.SCB]),
                                    start=True, stop=True)
                            nc.vector.tensor_copy(
                                stag[:, pb * cfg.PSB:(pb + 1) * cfg.PSB], bc_ps[:])
                        dst = cc_in if use_collective else bcp_o
                        nc.gpsimd.dma_start(
                            out=dst[row0:row0 + NCENT,
                                    q * cfg.QW:(q + 1) * cfg.QW],
                            in_=stag[:])

            # ---------- phase B (with phase A slices interleaved) ----------
            dots_ps = psA.tile([TC, SC], f32, name="dots_ps")
            zs_ps = psA.tile([1, SC], f32, name="zs_ps")
            ztp_ps = psA.tile([1, G * TC], f32, name="ztp_ps")

            spsg = (cfg.NCORES + cfg.NSG - 1) // cfg.NSG  # phase-A slices per sg

            for sg in range(cfg.NSG):
                tsg = tpool.tile([P, SG * G * TC], f32, name="tsg")
                csg = cpool.tile([P, SG * G * BL], f32, name="csg")
                nc.gpsimd.dma_start(out=tsg[:], in_=t_t[sg])
                nc.gpsimd.dma_start(out=csg[:], in_=c_t[sg])
                # u = t - c (in place into tsg), for both teacher views i=0,1
                tv = tsg[:].rearrange("p (k c) -> p k c", c=TC)
                cv = csg[:].rearrange("p (k j) -> p k j", j=BL)
                nc.vector.tensor_sub(tv[:, :, 0:BL], tv[:, :, 0:BL], cv)
                nc.vector.tensor_sub(tv[:, :, BL:TC], tv[:, :, BL:TC], cv)
                # qt = exp(u/temp - K)
                qt = qtpool.tile([P, SG * G * TC], dt_s, name="qt")
                nc.scalar.activation(qt[:], tsg[:], AF.Exp,
                                     bias=koff_t[:], scale=float(1.0 / temp))

                for gl in range(SG):
                    g = sg * SG + gl
                    s_tile = spool.tile([P, G * SC], dt_s, name="s_tile")
                    # issue on the scalar engine's HWDGE queue: parallel
                    # descriptor generation with the gpsimd-queue loads
                    nc.scalar.dma_start(out=s_tile[:], in_=s_t[g])
                    es = espool.tile([P, G * SC], dt_s, name="es")
                    nc.scalar.activation(es[:], s_tile[:], AF.Exp, scale=10.0)
                    first = g == 0
                    last = g == cfg.NGRP - 1
                    for c in range(G):
                        qsl = qt[:, (gl * G + c) * TC:(gl * G + c + 1) * TC]
                        nc.tensor.matmul(
                            dots_ps[:], mmview(qsl),
                            mmview(s_tile[:, c * SC:(c + 1) * SC]),
                            start=first and c == 0, stop=last and c == G - 1)
                        nc.tensor.matmul(
                            zs_ps[:], mmview(ones_t[:]),
                            mmview(es[:, c * SC:(c + 1) * SC]),
                            start=first and c == 0, stop=last and c == G - 1)
                    for h in range(G * TC // cfg.ZW):
                        nc.tensor.matmul(
                            ztp_ps[:, h * cfg.ZW:(h + 1) * cfg.ZW],
                            mmview(ones_t[:]),
                            mmview(qt[:, gl * G * TC + h * cfg.ZW:
                                       gl * G * TC + (h + 1) * cfg.ZW]),
                            start=first, stop=last)

                for si in range(sg * spsg, min((sg + 1) * spsg, cfg.NCORES)):
                    phase_a_slice(si)

            if use_collective:
                nc.gpsimd.collective_compute(
                    "ReduceScatter",
                    mybir.AluOpType.add,
                    replica_groups=[list(range(cfg.NCORES))],
                    ins=[cc_in[:].opt()],
                    outs=[cc_out[:].opt()],
                )

            wout = SC + SC + G * TC
            outst = opool.tile([TC, wout], f32, name="outst")
            nc.scalar.copy(outst[:TC, :SC], dots_ps[:])
            nc.scalar.copy(outst[:1, SC:2 * SC], zs_ps[:])
            nc.scalar.copy(outst[:1, 2 * SC:wout], ztp_ps[:])
            nc.gpsimd.dma_start(out=dots_o[:], in_=outst[:TC, :SC])
            nc.gpsimd.dma_start(out=zs_o[:], in_=outst[:1, SC:2 * SC])
            nc.gpsimd.dma_start(out=ztp_o[:], in_=outst[:1, 2 * SC:wout])

            # ---------- phase C: += 0.9*center slice, emit new_center slice ----------
            if use_collective:
                for half in range(2):
                    for q in range(cfg.DSH // cfg.CQW):
                        cs = slice(q * cfg.CQW, (q + 1) * cfg.CQW)
                        r0 = half * NCENT
                        cc_t = cph.tile([NCENT, cfg.CQW], f32, name="cc_t")
                        c09_t = cph.tile([NCENT, cfg.CQW], f32, name="c09_t")
                        nc.gpsimd.dma_start(out=cc_t[:], in_=cc_out[r0:r0 + NCENT, cs])
                        nc.gpsimd.dma_start(out=c09_t[:], in_=c09[r0:r0 + NCENT, cs])
                        nc.vector.tensor_add(cc_t[:], cc_t[:], c09_t[:])
                        nc.gpsimd.dma_start(out=nco_o[r0:r0 + NCENT, cs], in_=cc_t[:])

    nc.compile()
    return nc


# ----------------------------------------------------------------------------
# host-side sharding / assembly
# ----------------------------------------------------------------------------

def _to_bf16(a):
    import ml_dtypes
    return a.astype(ml_dtypes.bfloat16)


def make_in_maps(cfg: Cfg, student, teacher, center, targets):
    """Per-core input dicts. student/teacher/center are np.float32 full arrays."""
    D, BL, P, G = cfg.D, cfg.BL, cfg.P, cfg.G
    st3 = student.reshape(cfg.NCROPS, cfg.B, D)
    te3 = teacher.reshape(2, cfg.B, D)
    counts = np.bincount(targets, minlength=cfg.NCENT) * 2
    scale_t = (0.1 / np.maximum(counts, 1)).astype(np.float32)

    in_maps = []
    for k in range(cfg.NCORES):
        bsl = slice(k * BL, (k + 1) * BL)
        # transposed d-chunked layouts; col index within group g: c*cols + row
        S = st3[:, bsl, :].reshape(cfg.NCROPS * BL, D)   # rows v*BL+j
        S_t = np.ascontiguousarray(
            S.T.reshape(cfg.NGRP, G, P, cfg.SC).transpose(0, 2, 1, 3)
            .reshape(cfg.NGRP, P, G * cfg.SC))
        if cfg.bf16_student:
            S_t = _to_bf16(S_t)
        T = te3[:, bsl, :].reshape(2 * BL, D)            # rows i*BL+j
        SGG = cfg.SG * G
        T_t = np.ascontiguousarray(
            T.T.reshape(cfg.NSG, SGG, P, cfg.TC).transpose(0, 2, 1, 3)
            .reshape(cfg.NSG, P, SGG * cfg.TC))
        C = center[targets[bsl]]                          # [BL, D]
        C_t = np.ascontiguousarray(
            C.T.reshape(cfg.NSG, SGG, P, BL).transpose(0, 2, 1, 3)
            .reshape(cfg.NSG, P, SGG * BL))
        # teacher natural, packed into half-slices on the partition axis
        T_n = np.empty((cfg.NCORES, 2 * cfg.TC, cfg.DSH), np.float32)
        for s in range(cfg.NCORES):
            T_n[s, :cfg.TC] = T[:, s * cfg.DSL: s * cfg.DSL + cfg.DSH]
            T_n[s, cfg.TC:] = T[:, s * cfg.DSL + cfg.DSH:(s + 1) * cfg.DSL]
        # scaled one-hot, duplicated across both partition halves
        ohm = np.zeros((2 * cfg.TC, cfg.NCENT), np.float32)
        for i in range(2):
            for j in range(BL):
                t = targets[k * BL + j]
                ohm[i * BL + j, t] = scale_t[t]
        ohm[cfg.TC:] = ohm[:cfg.TC]
        # 0.9 * center for this core's d-slice, packed the same way
        c09 = np.empty((2 * cfg.NCENT, cfg.DSH), np.float32)
        c09[:cfg.NCENT] = 0.9 * center[:, k * cfg.DSL: k * cfg.DSL + cfg.DSH]
        c09[cfg.NCENT:] = 0.9 * center[:, k * cfg.DSL + cfg.DSH:(k + 1) * cfg.DSL]
        in_maps.append({
            "s_t": S_t, "t_t": T_t, "c_t": C_t, "t_n": T_n, "oh": ohm, "c09": c09,
        })
    return in_maps


def assemble(cfg: Cfg, results, center, temp, use_collective=True):
    BL = cfg.BL
    # ---- loss, assembled in float64 ----
    total = 0.0
    mask = np.ones((2, cfg.NCROPS))
    mask[0, 0] = 0.0
    mask[1, 1] = 0.0
    for k in range(cfg.NCORES):
        D4 = results[k]["dots_o"].astype(np.float64).reshape(2, BL, cfg.NCROPS, BL)
        d = D4[:, np.arange(BL), :, np.arange(BL)]        # [BL, 2, NCROPS]
        zs = results[k]["zs_o"].astype(np.float64).reshape(cfg.NCROPS, BL)
        ztp = results[k]["ztp_o"].astype(np.float64).reshape(cfg.G, 2, BL)
        zt = ztp.sum(axis=0)                              # [2, BL]
        term = (10.0 * d / zt.T[:, :, None]
                - np.log(zs).T[:, None, :])               # [BL, 2, NCROPS]
        total += (term * mask[None, :, :]).sum()
    loss = -total / ((2 * cfg.NCROPS - 2) * cfg.B)

    # ---- new_center ----
    full = np.empty((cfg.NCENT, cfg.D), np.float32)
    if use_collective:
        for k in range(cfg.NCORES):
            nco = results[k]["nco_o"]
            full[:, k * cfg.DSL: k * cfg.DSL + cfg.DSH] = nco[:cfg.NCENT]
            full[:, k * cfg.DSL + cfg.DSH:(k + 1) * cfg.DSL] = nco[cfg.NCENT:]
    else:
        bs = sum(results[k]["bcp_o"].astype(np.float64) for k in range(cfg.NCORES))
        for s in range(cfg.NCORES):
            blk = bs[s * 2 * cfg.NCENT:(s + 1) * 2 * cfg.NCENT]
            full[:, s * cfg.DSL: s * cfg.DSL + cfg.DSH] = blk[:cfg.NCENT]
            full[:, s * cfg.DSL + cfg.DSH:(s + 1) * cfg.DSL] = blk[cfg.NCENT:]
        full += np.float32(0.9) * center
    return np.float32(loss), full


_BUILT = {}


def _np(x, dtype=None):
    try:
        a = np.asarray(x)
    except Exception:
        import jax
        a = np.asarray(jax.device_get(x))
    return a.astype(dtype, copy=False) if dtype is not None else a


def kernel(student_output, teacher_output, center, targets, epoch, _want_trace=False):
    bass, bacc, mybir, tile, bass_utils = _import_concourse()
    student = _np(student_output, np.float32)
    teacher = _np(teacher_output, np.float32)
    center_np = _np(center, np.float32)
    targets_np = _np(targets).astype(np.int64)
    temp = _teacher_temp(int(_np(epoch)))

    cfg = Cfg(bf16_student=bool(int(os.environ.get("DINO_BF16", "1"))))
    use_cc = bool(int(os.environ.get("DINO_COLLECTIVE", "1")))
    key = (temp, cfg.bf16_student, use_cc)
    if key not in _BUILT:
        _BUILT[key] = build_kernel(cfg, temp, use_collective=use_cc)
    nc = _BUILT[key]

    in_maps = make_in_maps(cfg, student, teacher, center_np, targets_np)
    kw = {}
    if _want_trace:
        kw = dict(trace=True, trace_cores=list(range(cfg.NCORES)))
    res = bass_utils.run_bass_kernel_spmd(
        nc, in_maps, core_ids=list(range(cfg.NCORES)), **kw)
    out = assemble(cfg, res.results, center_np, temp, use_collective=use_cc)
    if _want_trace:
        return out, res
    return out


# revision 28
# speedup vs baseline: 1.2338x; 1.2338x over previous
"""Trainium2 Bass kernel for DINO loss (nn_DINOLoss_44083544326419).

kernel(**inputs) takes FULL unsharded inputs and returns (total_loss, new_center),
matching the reference nn.Module. Strategy:

- batch dim (B=256) sharded across 8 NeuronCores (32 samples/core)
- per core, heavy math runs in a d-chunked transposed layout
  ([128 d-partitions x row-columns]); full-row reductions become PE matmuls
  against a ones-vector, accumulated in PSUM across all 512 d-chunks:
    * dots[(i,b),(v,b')] = sum_d exp((t-c)/temp - K) * student
    * zs = sum_d exp(10*student)   (student softmax denominator)
    * ztp = sum_d qt               (teacher softmax denominator, per chunk-phase)
  student log-softmax needs no max-subtraction (|10 s|max ~ 55 fits f32 exp);
  teacher softmax uses a fixed offset K=100 (max u ~ 110 -> exp(u-K) <= e^10).
- center update: scatter-add as a one-hot matmul (one-hot columns pre-scaled by
  0.1/max(2*counts,1) on host), summed across cores with an on-device
  ReduceScatter; each core adds 0.9*center for its d-slice and writes that
  slice of new_center.
- tiny per-(i,v,b) scalar assembly (logs, masking, means) happens on host in
  float64 from the per-core reduction outputs (~85 KB/core).
"""

import os
import sys

import numpy as np

KOFF = 100.0  # teacher exp offset
WARMUP_T = 0.04
TEACHER_T = 0.07
WARMUP_EPOCHS = 30
NEPOCHS = 100


def _teacher_temp(epoch: int) -> float:
    sched = np.concatenate(
        (np.linspace(WARMUP_T, TEACHER_T, WARMUP_EPOCHS),
         np.ones(NEPOCHS - WARMUP_EPOCHS) * TEACHER_T))
    return float(sched[int(epoch)])


def _import_concourse():
    try:
        import concourse.bass  # noqa: F401
    except ImportError:
        for p in ("/opt/trn_rl_repo", "/root/.axon_site/_ro/trn_rl_repo"):
            if os.path.isdir(p) and p not in sys.path:
                sys.path.insert(0, p)
    import concourse.bass as bass
    import concourse.bacc as bacc
    import concourse.mybir as mybir
    import concourse.tile as tile
    from concourse import bass_utils
    return bass, bacc, mybir, tile, bass_utils


class Cfg:
    """Problem geometry. Defaults = the real problem; overridable for sim tests."""

    def __init__(self, D=65536, BL=32, NCROPS=10, NCENT=51, NCORES=8, G=16, SG=2,
                 bf16_student=True):
        self.D = D                    # feature dim
        self.BL = BL                  # local batch per core
        self.NCROPS = NCROPS
        self.NCENT = NCENT
        self.NCORES = NCORES
        self.B = BL * NCORES          # global batch
        self.P = 128                  # d-chunk partition size
        assert D % (self.P * NCORES * 2) == 0
        self.NCHUNK = D // self.P     # number of d-chunks
        self.G = G                    # chunks per student group
        assert self.NCHUNK % G == 0
        self.NGRP = self.NCHUNK // G
        self.SG = SG                  # student groups per teacher supergroup
        assert self.NGRP % SG == 0
        self.NSG = self.NGRP // SG
        self.TC = 2 * BL              # teacher cols per chunk
        self.SC = NCROPS * BL         # student cols per chunk
        assert 2 * self.TC <= 128
        self.DSL = D // NCORES        # d-slice width per core (center output)
        self.DSH = self.DSL // 2      # packed half-slice width
        self.SCB = min(512, self.DSH)          # scatter matmul free-dim block
        assert self.DSH % self.SCB == 0
        self.QW = min(4096, self.DSH)          # scatter staging width
        assert self.DSH % self.QW == 0
        self.PSB = min(1024, self.QW)          # scatter psum tile width
        assert self.QW % self.PSB == 0 and self.PSB % self.SCB == 0
        self.CQW = min(2048, self.DSH)         # phase-C tile width
        assert self.DSH % self.CQW == 0
        self.ZW = min(512, self.G * self.TC)   # ztp matmul slice width
        assert (self.G * self.TC) % self.ZW == 0
        self.bf16_student = bf16_student


def build_kernel(cfg: Cfg, temp: float, use_collective=True):
    """Builds, schedules and compiles the SPMD kernel; returns the Bacc module."""
    bass, bacc, mybir, tile, bass_utils = _import_concourse()
    dt = mybir.dt
    f32 = dt.float32
    # matmul-feeding dtype: bf16, or float32r (f32 bits, 1 cyc/row on PE when
    # the moving dim >= 256; walrus requires producers typed f32r end-to-end)
    dt_s = dt.bfloat16 if cfg.bf16_student else dt.float32r
    dt_r = dt.float32r
    AF = mybir.ActivationFunctionType

    def mmview(ap):
        return ap

    nc = bacc.Bacc("TRN2", target_bir_lowering=False, debug=False,
                   num_devices=cfg.NCORES)

    P, G, SG, TC, SC, NCENT, BL = cfg.P, cfg.G, cfg.SG, cfg.TC, cfg.SC, cfg.NCENT, cfg.BL

    s_t = nc.dram_tensor("s_t", [cfg.NGRP, P, G * SC], dt_s, kind="ExternalInput")
    t_t = nc.dram_tensor("t_t", [cfg.NSG, P, SG * G * TC], f32, kind="ExternalInput")
    c_t = nc.dram_tensor("c_t", [cfg.NSG, P, SG * G * BL], f32, kind="ExternalInput")
    t_n = nc.dram_tensor("t_n", [cfg.NCORES, 2 * TC, cfg.DSH], dt_r,
                         kind="ExternalInput")
    oh = nc.dram_tensor("oh", [2 * TC, NCENT], dt_r, kind="ExternalInput")
    c09 = nc.dram_tensor("c09", [2 * NCENT, cfg.DSH], f32, kind="ExternalInput")

    dots_o = nc.dram_tensor("dots_o", [TC, SC], f32, kind="ExternalOutput")
    zs_o = nc.dram_tensor("zs_o", [1, SC], f32, kind="ExternalOutput")
    ztp_o = nc.dram_tensor("ztp_o", [1, G * TC], f32, kind="ExternalOutput")
    if use_collective:
        nco_o = nc.dram_tensor("nco_o", [2 * NCENT, cfg.DSH], f32,
                               kind="ExternalOutput")
    else:
        bcp_o = nc.dram_tensor("bcp_o", [cfg.NCORES * 2 * NCENT, cfg.DSH], f32,
                               kind="ExternalOutput")

    with tile.TileContext(nc) as tc:
        with (
            tc.tile_pool(name="const", bufs=1) as constp,
            tc.tile_pool(name="tn", bufs=1) as tnp,
            tc.tile_pool(name="stag", bufs=2) as stagp,
            tc.tile_pool(name="spool", bufs=3) as spool,
            tc.tile_pool(name="espool", bufs=3) as espool,
            tc.tile_pool(name="tpool", bufs=2) as tpool,
            tc.tile_pool(name="cpool", bufs=1) as cpool,
            tc.tile_pool(name="qtpool", bufs=2) as qtpool,
            tc.tile_pool(name="opool", bufs=1) as opool,
            tc.tile_pool(name="cph", bufs=1) as cph,
            tc.tile_pool(name="psA", bufs=1, space="PSUM") as psA,
            tc.tile_pool(name="psS", bufs=2, space="PSUM") as psS,
            tc.tile_pool(name="dram", bufs=1, space="DRAM") as dram,
        ):
            ones_t = constp.tile([P, 1], dt_s, name="ones_t")
            nc.gpsimd.memset(ones_t[:], 1.0)
            koff_t = constp.tile([P, 1], f32, name="koff_t")
            nc.gpsimd.memset(koff_t[:], -KOFF)
            oh_t = constp.tile([2 * TC, NCENT], dt_r, name="oh_t")
            nc.gpsimd.dma_start(out=oh_t[:], in_=oh[:])

            if use_collective:
                cc_in = dram.tile([cfg.NCORES * 2 * NCENT, cfg.DSH], f32,
                                  name="cc_in")
                cc_out = dram.tile([2 * NCENT, cfg.DSH], f32, name="cc_out")

            # ---------- phase A slice emitter: scatter-add via one-hot matmul ----
            def phase_a_slice(s):
                tn_t = tnp.tile([2 * TC, cfg.DSH], dt_r, name="tn_t")
                nc.gpsimd.dma_start(out=tn_t[:], in_=t_n[s])
                for half in range(2):
                    pr0 = half * TC
                    row0 = s * 2 * NCENT + half * NCENT
                    for q in range(cfg.DSH // cfg.QW):
                        stag = stagp.tile([NCENT, cfg.QW], f32, name="stag")
                        for pb in range(cfg.QW // cfg.PSB):
                            bc_ps = psS.tile([NCENT, cfg.PSB], f32, name="bc_ps")
                            for m in range(cfg.PSB // cfg.SCB):
                                col = q * cfg.QW + pb * cfg.PSB + m * cfg.SCB
                                nc.tensor.matmul(
                                    bc_ps[:, m * cfg.SCB:(m + 1) * cfg.SCB],
                                    mmview(oh_t[pr0:pr0 + TC, :]),
                                    mmview(tn_t[pr0:pr0 + TC, col:col + cfg.SCB]),
                                    start=True, stop=True)
                            nc.vector.tensor_copy(
                                stag[:, pb * cfg.PSB:(pb + 1) * cfg.PSB], bc_ps[:])
                        dst = cc_in if use_collective else bcp_o
                        nc.gpsimd.dma_start(
                            out=dst[row0:row0 + NCENT,
                                    q * cfg.QW:(q + 1) * cfg.QW],
                            in_=stag[:])

            # ---------- phase B (with phase A slices interleaved) ----------
            dots_ps = psA.tile([TC, SC], f32, name="dots_ps")
            zs_ps = psA.tile([1, SC], f32, name="zs_ps")
            ztp_ps = psA.tile([1, G * TC], f32, name="ztp_ps")

            spsg = (cfg.NCORES + cfg.NSG - 1) // cfg.NSG  # phase-A slices per sg

            for sg in range(cfg.NSG):
                tsg = tpool.tile([P, SG * G * TC], f32, name="tsg")
                csg = cpool.tile([P, SG * G * BL], f32, name="csg")
                nc.gpsimd.dma_start(out=tsg[:], in_=t_t[sg])
                nc.gpsimd.dma_start(out=csg[:], in_=c_t[sg])
                # u = t - c (in place into tsg), for both teacher views i=0,1
                tv = tsg[:].rearrange("p (k c) -> p k c", c=TC)
                cv = csg[:].rearrange("p (k j) -> p k j", j=BL)
                nc.vector.tensor_sub(tv[:, :, 0:BL], tv[:, :, 0:BL], cv)
                nc.vector.tensor_sub(tv[:, :, BL:TC], tv[:, :, BL:TC], cv)
                # qt = exp(u/temp - K)
                qt = qtpool.tile([P, SG * G * TC], dt_s, name="qt")
                nc.scalar.activation(qt[:], tsg[:], AF.Exp,
                                     bias=koff_t[:], scale=float(1.0 / temp))

                for gl in range(SG):
                    g = sg * SG + gl
                    s_tile = spool.tile([P, G * SC], dt_s, name="s_tile")
                    nc.gpsimd.dma_start(out=s_tile[:], in_=s_t[g])
                    es = espool.tile([P, G * SC], dt_s, name="es")
                    nc.scalar.activation(es[:], s_tile[:], AF.Exp, scale=10.0)
                    first = g == 0
                    last = g == cfg.NGRP - 1
                    for c in range(G):
                        qsl = qt[:, (gl * G + c) * TC:(gl * G + c + 1) * TC]
                        nc.tensor.matmul(
                            dots_ps[:], mmview(qsl),
                            mmview(s_tile[:, c * SC:(c + 1) * SC]),
                            start=first and c == 0, stop=last and c == G - 1)
                        nc.tensor.matmul(
                            zs_ps[:], mmview(ones_t[:]),
                            mmview(es[:, c * SC:(c + 1) * SC]),
                            start=first and c == 0, stop=last and c == G - 1)
                    for h in range(G * TC // cfg.ZW):
                        nc.tensor.matmul(
                            ztp_ps[:, h * cfg.ZW:(h + 1) * cfg.ZW],
                            mmview(ones_t[:]),
                            mmview(qt[:, gl * G * TC + h * cfg.ZW:
                                       gl * G * TC + (h + 1) * cfg.ZW]),
                            start=first, stop=last)

                for si in range(sg * spsg, min((sg + 1) * spsg, cfg.NCORES)):
                    phase_a_slice(si)

            if use_collective:
                nc.gpsimd.collective_compute(
                    "ReduceScatter",
                    mybir.AluOpType.add,
                    replica_groups=[list(range(cfg.NCORES))],
                    ins=[cc_in[:].opt()],
                    outs=[cc_out[:].opt()],
                )

            wout = SC + SC + G * TC
            outst = opool.tile([TC, wout], f32, name="outst")
            nc.scalar.copy(outst[:TC, :SC], dots_ps[:])
            nc.scalar.copy(outst[:1, SC:2 * SC], zs_ps[:])
            nc.scalar.copy(outst[:1, 2 * SC:wout], ztp_ps[:])
            nc.gpsimd.dma_start(out=dots_o[:], in_=outst[:TC, :SC])
            nc.gpsimd.dma_start(out=zs_o[:], in_=outst[:1, SC:2 * SC])
            nc.gpsimd.dma_start(out=ztp_o[:], in_=outst[:1, 2 * SC:wout])

            # ---------- phase C: += 0.9*center slice, emit new_center slice ----------
            if use_collective:
                for half in range(2):
                    for q in range(cfg.DSH // cfg.CQW):
                        cs = slice(q * cfg.CQW, (q + 1) * cfg.CQW)
                        r0 = half * NCENT
                        cc_t = cph.tile([NCENT, cfg.CQW], f32, name="cc_t")
                        c09_t = cph.tile([NCENT, cfg.CQW], f32, name="c09_t")
                        nc.gpsimd.dma_start(out=cc_t[:], in_=cc_out[r0:r0 + NCENT, cs])
                        nc.gpsimd.dma_start(out=c09_t[:], in_=c09[r0:r0 + NCENT, cs])
                        nc.vector.tensor_add(cc_t[:], cc_t[:], c09_t[:])
                        nc.gpsimd.dma_start(out=nco_o[r0:r0 + NCENT, cs], in_=cc_t[:])

    nc.compile()
    return nc


# ----------------------------------------------------------------------------
# host-side sharding / assembly
# ----------------------------------------------------------------------------

def _to_bf16(a):
    import ml_dtypes
    return a.astype(ml_dtypes.bfloat16)


def make_in_maps(cfg: Cfg, student, teacher, center, targets):
    """Per-core input dicts. student/teacher/center are np.float32 full arrays."""
    D, BL, P, G = cfg.D, cfg.BL, cfg.P, cfg.G
    st3 = student.reshape(cfg.NCROPS, cfg.B, D)
    te3 = teacher.reshape(2, cfg.B, D)
    counts = np.bincount(targets, minlength=cfg.NCENT) * 2
    scale_t = (0.1 / np.maximum(counts, 1)).astype(np.float32)

    in_maps = []
    for k in range(cfg.NCORES):
        bsl = slice(k * BL, (k + 1) * BL)
        # transposed d-chunked layouts; col index within group g: c*cols + row
        S = st3[:, bsl, :].reshape(cfg.NCROPS * BL, D)   # rows v*BL+j
        S_t = np.ascontiguousarray(
            S.T.reshape(cfg.NGRP, G, P, cfg.SC).transpose(0, 2, 1, 3)
            .reshape(cfg.NGRP, P, G * cfg.SC))
        if cfg.bf16_student:
            S_t = _to_bf16(S_t)
        T = te3[:, bsl, :].reshape(2 * BL, D)            # rows i*BL+j
        SGG = cfg.SG * G
        T_t = np.ascontiguousarray(
            T.T.reshape(cfg.NSG, SGG, P, cfg.TC).transpose(0, 2, 1, 3)
            .reshape(cfg.NSG, P, SGG * cfg.TC))
        C = center[targets[bsl]]                          # [BL, D]
        C_t = np.ascontiguousarray(
            C.T.reshape(cfg.NSG, SGG, P, BL).transpose(0, 2, 1, 3)
            .reshape(cfg.NSG, P, SGG * BL))
        # teacher natural, packed into half-slices on the partition axis
        T_n = np.empty((cfg.NCORES, 2 * cfg.TC, cfg.DSH), np.float32)
        for s in range(cfg.NCORES):
            T_n[s, :cfg.TC] = T[:, s * cfg.DSL: s * cfg.DSL + cfg.DSH]
            T_n[s, cfg.TC:] = T[:, s * cfg.DSL + cfg.DSH:(s + 1) * cfg.DSL]
        # scaled one-hot, duplicated across both partition halves
        ohm = np.zeros((2 * cfg.TC, cfg.NCENT), np.float32)
        for i in range(2):
            for j in range(BL):
                t = targets[k * BL + j]
                ohm[i * BL + j, t] = scale_t[t]
        ohm[cfg.TC:] = ohm[:cfg.TC]
        # 0.9 * center for this core's d-slice, packed the same way
        c09 = np.empty((2 * cfg.NCENT, cfg.DSH), np.float32)
        c09[:cfg.NCENT] = 0.9 * center[:, k * cfg.DSL: k * cfg.DSL + cfg.DSH]
        c09[cfg.NCENT:] = 0.9 * center[:, k * cfg.DSL + cfg.DSH:(k + 1) * cfg.DSL]
        in_maps.append({
            "s_t": S_t, "t_t": T_t, "c_t": C_t, "t_n": T_n, "oh": ohm, "c09": c09,
        })
    return in_maps


def assemble(cfg: Cfg, results, center, temp, use_collective=True):
    BL = cfg.BL
    # ---- loss, assembled in float64 ----
    total = 0.0
    mask = np.ones((2, cfg.NCROPS))
    mask[0, 0] = 0.0
    mask[1, 1] = 0.0
    for k in range(cfg.NCORES):
        D4 = results[k]["dots_o"].astype(np.float64).reshape(2, BL, cfg.NCROPS, BL)
        d = D4[:, np.arange(BL), :, np.arange(BL)]        # [BL, 2, NCROPS]
        zs = results[k]["zs_o"].astype(np.float64).reshape(cfg.NCROPS, BL)
        ztp = results[k]["ztp_o"].astype(np.float64).reshape(cfg.G, 2, BL)
        zt = ztp.sum(axis=0)                              # [2, BL]
        term = (10.0 * d / zt.T[:, :, None]
                - np.log(zs).T[:, None, :])               # [BL, 2, NCROPS]
        total += (term * mask[None, :, :]).sum()
    loss = -total / ((2 * cfg.NCROPS - 2) * cfg.B)

    # ---- new_center ----
    full = np.empty((cfg.NCENT, cfg.D), np.float32)
    if use_collective:
        for k in range(cfg.NCORES):
            nco = results[k]["nco_o"]
            full[:, k * cfg.DSL: k * cfg.DSL + cfg.DSH] = nco[:cfg.NCENT]
            full[:, k * cfg.DSL + cfg.DSH:(k + 1) * cfg.DSL] = nco[cfg.NCENT:]
    else:
        bs = sum(results[k]["bcp_o"].astype(np.float64) for k in range(cfg.NCORES))
        for s in range(cfg.NCORES):
            blk = bs[s * 2 * cfg.NCENT:(s + 1) * 2 * cfg.NCENT]
            full[:, s * cfg.DSL: s * cfg.DSL + cfg.DSH] = blk[:cfg.NCENT]
            full[:, s * cfg.DSL + cfg.DSH:(s + 1) * cfg.DSL] = blk[cfg.NCENT:]
        full += np.float32(0.9) * center
    return np.float32(loss), full


_BUILT = {}


def kernel(student_output, teacher_output, center, targets, epoch, _want_trace=False):
    bass, bacc, mybir, tile, bass_utils = _import_concourse()
    student = np.asarray(student_output, np.float32)
    teacher = np.asarray(teacher_output, np.float32)
    center_np = np.asarray(center, np.float32)
    targets_np = np.asarray(targets).astype(np.int64)
    temp = _teacher_temp(int(epoch))

    cfg = Cfg(bf16_student=bool(int(os.environ.get("DINO_BF16", "1"))))
    use_cc = bool(int(os.environ.get("DINO_COLLECTIVE", "1")))
    key = (temp, cfg.bf16_student, use_cc)
    if key not in _BUILT:
        _BUILT[key] = build_kernel(cfg, temp, use_collective=use_cc)
    nc = _BUILT[key]

    in_maps = make_in_maps(cfg, student, teacher, center_np, targets_np)
    kw = {}
    if _want_trace:
        kw = dict(trace=True, trace_cores=list(range(cfg.NCORES)))
    res = bass_utils.run_bass_kernel_spmd(
        nc, in_maps, core_ids=list(range(cfg.NCORES)), **kw)
    out = assemble(cfg, res.results, center_np, temp, use_collective=use_cc)
    if _want_trace:
        return out, res
    return out


# revision 29
# speedup vs baseline: 1.7885x; 1.4496x over previous
"""Trainium2 Bass kernel for DINO loss (nn_DINOLoss_44083544326419).

kernel(**inputs) takes FULL unsharded inputs and returns (total_loss, new_center),
matching the reference nn.Module. Strategy:

- batch dim (B=256) sharded across 8 NeuronCores (32 samples/core)
- per core, heavy math runs in a d-chunked transposed layout
  ([128 d-partitions x row-columns]); full-row reductions become PE matmuls
  against a ones-vector, accumulated in PSUM across all 512 d-chunks:
    * dots[(i,b),(v,b')] = sum_d exp((t-c)/temp - K) * student
    * zs = sum_d exp(10*student)   (student softmax denominator)
    * ztp = sum_d qt               (teacher softmax denominator, per chunk-phase)
  student log-softmax needs no max-subtraction (|10 s|max ~ 55 fits f32 exp);
  teacher softmax uses a fixed offset K=100 (max u ~ 110 -> exp(u-K) <= e^10).
- center update: scatter-add as a one-hot matmul (one-hot columns pre-scaled by
  0.1/max(2*counts,1) on host), summed across cores with an on-device
  ReduceScatter; each core adds 0.9*center for its d-slice and writes that
  slice of new_center.
- tiny per-(i,v,b) scalar assembly (logs, masking, means) happens on host in
  float64 from the per-core reduction outputs (~85 KB/core).
"""

import os
import sys

import numpy as np

KOFF = 100.0  # teacher exp offset
WARMUP_T = 0.04
TEACHER_T = 0.07
WARMUP_EPOCHS = 30
NEPOCHS = 100


def _teacher_temp(epoch: int) -> float:
    sched = np.concatenate(
        (np.linspace(WARMUP_T, TEACHER_T, WARMUP_EPOCHS),
         np.ones(NEPOCHS - WARMUP_EPOCHS) * TEACHER_T))
    return float(sched[int(epoch)])


def _import_concourse():
    try:
        import concourse.bass  # noqa: F401
    except ImportError:
        for p in ("/opt/trn_rl_repo", "/root/.axon_site/_ro/trn_rl_repo"):
            if os.path.isdir(p) and p not in sys.path:
                sys.path.insert(0, p)
    import concourse.bass as bass
    import concourse.bacc as bacc
    import concourse.mybir as mybir
    import concourse.tile as tile
    from concourse import bass_utils
    return bass, bacc, mybir, tile, bass_utils


class Cfg:
    """Problem geometry. Defaults = the real problem; overridable for sim tests."""

    def __init__(self, D=65536, BL=32, NCROPS=10, NCENT=51, NCORES=8, G=16, SG=2,
                 bf16_student=True):
        self.D = D                    # feature dim
        self.BL = BL                  # local batch per core
        self.NCROPS = NCROPS
        self.NCENT = NCENT
        self.NCORES = NCORES
        self.B = BL * NCORES          # global batch
        self.P = 128                  # d-chunk partition size
        assert D % (self.P * NCORES * 2) == 0
        self.NCHUNK = D // self.P     # number of d-chunks
        self.G = G                    # chunks per student group
        assert self.NCHUNK % G == 0
        self.NGRP = self.NCHUNK // G
        self.SG = SG                  # student groups per teacher supergroup
        assert self.NGRP % SG == 0
        self.NSG = self.NGRP // SG
        self.TC = 2 * BL              # teacher cols per chunk
        self.SC = NCROPS * BL         # student cols per chunk
        assert 2 * self.TC <= 128
        self.DSL = D // NCORES        # d-slice width per core (center output)
        self.DSH = self.DSL // 2      # packed half-slice width
        self.SCB = min(512, self.DSH)          # scatter matmul free-dim block
        assert self.DSH % self.SCB == 0
        self.QW = min(4096, self.DSH)          # scatter staging width
        assert self.DSH % self.QW == 0
        self.PSB = min(1024, self.QW)          # scatter psum tile width
        assert self.QW % self.PSB == 0 and self.PSB % self.SCB == 0
        self.CQW = min(2048, self.DSH)         # phase-C tile width
        assert self.DSH % self.CQW == 0
        self.ZW = min(512, self.G * self.TC)   # ztp matmul slice width
        assert (self.G * self.TC) % self.ZW == 0
        self.bf16_student = bf16_student


def build_kernel(cfg: Cfg, temp: float, use_collective=True):
    """Builds, schedules and compiles the SPMD kernel; returns the Bacc module."""
    bass, bacc, mybir, tile, bass_utils = _import_concourse()
    dt = mybir.dt
    f32 = dt.float32
    # matmul-feeding dtype: bf16, or float32r (f32 bits, 1 cyc/row on PE when
    # the moving dim >= 256; walrus requires producers typed f32r end-to-end)
    dt_s = dt.bfloat16 if cfg.bf16_student else dt.float32r
    dt_r = dt.float32r
    AF = mybir.ActivationFunctionType

    def mmview(ap):
        return ap

    nc = bacc.Bacc("TRN2", target_bir_lowering=False, debug=False,
                   num_devices=cfg.NCORES)

    P, G, SG, TC, SC, NCENT, BL = cfg.P, cfg.G, cfg.SG, cfg.TC, cfg.SC, cfg.NCENT, cfg.BL

    s_t = nc.dram_tensor("s_t", [cfg.NGRP, P, G * SC], dt_s, kind="ExternalInput")
    t_t = nc.dram_tensor("t_t", [cfg.NSG, P, SG * G * TC], f32, kind="ExternalInput")
    c_t = nc.dram_tensor("c_t", [cfg.NSG, P, SG * G * BL], f32, kind="ExternalInput")
    t_n = nc.dram_tensor("t_n", [cfg.NCORES, 2 * TC, cfg.DSH], dt_r,
                         kind="ExternalInput")
    oh = nc.dram_tensor("oh", [2 * TC, NCENT], dt_r, kind="ExternalInput")
    c09 = nc.dram_tensor("c09", [2 * NCENT, cfg.DSH], f32, kind="ExternalInput")

    dots_o = nc.dram_tensor("dots_o", [TC, SC], f32, kind="ExternalOutput")
    zs_o = nc.dram_tensor("zs_o", [1, SC], f32, kind="ExternalOutput")
    ztp_o = nc.dram_tensor("ztp_o", [1, G * TC], f32, kind="ExternalOutput")
    if use_collective:
        nco_o = nc.dram_tensor("nco_o", [2 * NCENT, cfg.DSH], f32,
                               kind="ExternalOutput")
    else:
        bcp_o = nc.dram_tensor("bcp_o", [cfg.NCORES * 2 * NCENT, cfg.DSH], f32,
                               kind="ExternalOutput")

    with tile.TileContext(nc) as tc:
        with (
            tc.tile_pool(name="const", bufs=1) as constp,
            tc.tile_pool(name="tn", bufs=2) as tnp,
            tc.tile_pool(name="stag", bufs=2) as stagp,
            tc.tile_pool(name="spool", bufs=2) as spool,
            tc.tile_pool(name="espool", bufs=2) as espool,
            tc.tile_pool(name="tpool", bufs=2) as tpool,
            tc.tile_pool(name="cpool", bufs=2) as cpool,
            tc.tile_pool(name="qtpool", bufs=2) as qtpool,
            tc.tile_pool(name="opool", bufs=1) as opool,
            tc.tile_pool(name="cph", bufs=1) as cph,
            tc.tile_pool(name="psA", bufs=1, space="PSUM") as psA,
            tc.tile_pool(name="psS", bufs=2, space="PSUM") as psS,
            tc.tile_pool(name="dram", bufs=1, space="DRAM") as dram,
        ):
            ones_t = constp.tile([P, 1], dt_s, name="ones_t")
            nc.gpsimd.memset(ones_t[:], 1.0)
            koff_t = constp.tile([P, 1], f32, name="koff_t")
            nc.gpsimd.memset(koff_t[:], -KOFF)
            oh_t = constp.tile([2 * TC, NCENT], dt_r, name="oh_t")
            nc.gpsimd.dma_start(out=oh_t[:], in_=oh[:])

            if use_collective:
                cc_in = dram.tile([cfg.NCORES * 2 * NCENT, cfg.DSH], f32,
                                  name="cc_in")
                cc_out = dram.tile([2 * NCENT, cfg.DSH], f32, name="cc_out")

            # ---------- phase A slice emitter: scatter-add via one-hot matmul ----
            def phase_a_slice(s):
                tn_t = tnp.tile([2 * TC, cfg.DSH], dt_r, name="tn_t")
                nc.gpsimd.dma_start(out=tn_t[:], in_=t_n[s])
                for half in range(2):
                    pr0 = half * TC
                    row0 = s * 2 * NCENT + half * NCENT
                    for q in range(cfg.DSH // cfg.QW):
                        stag = stagp.tile([NCENT, cfg.QW], f32, name="stag")
                        for pb in range(cfg.QW // cfg.PSB):
                            bc_ps = psS.tile([NCENT, cfg.PSB], f32, name="bc_ps")
                            for m in range(cfg.PSB // cfg.SCB):
                                col = q * cfg.QW + pb * cfg.PSB + m * cfg.SCB
                                nc.tensor.matmul(
                                    bc_ps[:, m * cfg.SCB:(m + 1) * cfg.SCB],
                                    mmview(oh_t[pr0:pr0 + TC, :]),
                                    mmview(tn_t[pr0:pr0 + TC, col:col + cfg.SCB]),
                                    start=True, stop=True)
                            nc.vector.tensor_copy(
                                stag[:, pb * cfg.PSB:(pb + 1) * cfg.PSB], bc_ps[:])
                        dst = cc_in if use_collective else bcp_o
                        nc.gpsimd.dma_start(
                            out=dst[row0:row0 + NCENT,
                                    q * cfg.QW:(q + 1) * cfg.QW],
                            in_=stag[:])

            # ---------- phase B (with phase A slices interleaved) ----------
            dots_ps = psA.tile([TC, SC], f32, name="dots_ps")
            zs_ps = psA.tile([1, SC], f32, name="zs_ps")
            ztp_ps = psA.tile([1, G * TC], f32, name="ztp_ps")

            spsg = (cfg.NCORES + cfg.NSG - 1) // cfg.NSG  # phase-A slices per sg

            for sg in range(cfg.NSG):
                tsg = tpool.tile([P, SG * G * TC], f32, name="tsg")
                csg = cpool.tile([P, SG * G * BL], f32, name="csg")
                nc.gpsimd.dma_start(out=tsg[:], in_=t_t[sg])
                nc.gpsimd.dma_start(out=csg[:], in_=c_t[sg])
                # u = t - c (in place into tsg), for both teacher views i=0,1
                tv = tsg[:].rearrange("p (k c) -> p k c", c=TC)
                cv = csg[:].rearrange("p (k j) -> p k j", j=BL)
                nc.vector.tensor_sub(tv[:, :, 0:BL], tv[:, :, 0:BL], cv)
                nc.vector.tensor_sub(tv[:, :, BL:TC], tv[:, :, BL:TC], cv)
                # qt = exp(u/temp - K)
                qt = qtpool.tile([P, SG * G * TC], dt_s, name="qt")
                nc.scalar.activation(qt[:], tsg[:], AF.Exp,
                                     bias=koff_t[:], scale=float(1.0 / temp))

                for gl in range(SG):
                    g = sg * SG + gl
                    s_tile = spool.tile([P, G * SC], dt_s, name="s_tile")
                    # issue on the scalar engine's HWDGE queue: parallel
                    # descriptor generation with the gpsimd-queue loads
                    nc.scalar.dma_start(out=s_tile[:], in_=s_t[g])
                    es = espool.tile([P, G * SC], dt_s, name="es")
                    nc.scalar.activation(es[:], s_tile[:], AF.Exp, scale=10.0)
                    first = g == 0
                    last = g == cfg.NGRP - 1
                    for c in range(G):
                        qsl = qt[:, (gl * G + c) * TC:(gl * G + c + 1) * TC]
                        nc.tensor.matmul(
                            dots_ps[:], mmview(qsl),
                            mmview(s_tile[:, c * SC:(c + 1) * SC]),
                            start=first and c == 0, stop=last and c == G - 1)
                        nc.tensor.matmul(
                            zs_ps[:], mmview(ones_t[:]),
                            mmview(es[:, c * SC:(c + 1) * SC]),
                            start=first and c == 0, stop=last and c == G - 1)
                    for h in range(G * TC // cfg.ZW):
                        nc.tensor.matmul(
                            ztp_ps[:, h * cfg.ZW:(h + 1) * cfg.ZW],
                            mmview(ones_t[:]),
                            mmview(qt[:, gl * G * TC + h * cfg.ZW:
                                       gl * G * TC + (h + 1) * cfg.ZW]),
                            start=first, stop=last)

                for si in range(sg * spsg, min((sg + 1) * spsg, cfg.NCORES)):
                    phase_a_slice(si)

            if use_collective:
                nc.gpsimd.collective_compute(
                    "ReduceScatter",
                    mybir.AluOpType.add,
                    replica_groups=[list(range(cfg.NCORES))],
                    ins=[cc_in[:].opt()],
                    outs=[cc_out[:].opt()],
                )

            wout = SC + SC + G * TC
            outst = opool.tile([TC, wout], f32, name="outst")
            nc.scalar.copy(outst[:TC, :SC], dots_ps[:])
            nc.scalar.copy(outst[:1, SC:2 * SC], zs_ps[:])
            nc.scalar.copy(outst[:1, 2 * SC:wout], ztp_ps[:])
            nc.gpsimd.dma_start(out=dots_o[:], in_=outst[:TC, :SC])
            nc.gpsimd.dma_start(out=zs_o[:], in_=outst[:1, SC:2 * SC])
            nc.gpsimd.dma_start(out=ztp_o[:], in_=outst[:1, 2 * SC:wout])

            # ---------- phase C: += 0.9*center slice, emit new_center slice ----------
            if use_collective:
                for half in range(2):
                    for q in range(cfg.DSH // cfg.CQW):
                        cs = slice(q * cfg.CQW, (q + 1) * cfg.CQW)
                        r0 = half * NCENT
                        cc_t = cph.tile([NCENT, cfg.CQW], f32, name="cc_t")
                        c09_t = cph.tile([NCENT, cfg.CQW], f32, name="c09_t")
                        nc.gpsimd.dma_start(out=cc_t[:], in_=cc_out[r0:r0 + NCENT, cs])
                        nc.gpsimd.dma_start(out=c09_t[:], in_=c09[r0:r0 + NCENT, cs])
                        nc.vector.tensor_add(cc_t[:], cc_t[:], c09_t[:])
                        nc.gpsimd.dma_start(out=nco_o[r0:r0 + NCENT, cs], in_=cc_t[:])

    nc.compile()
    return nc


# ----------------------------------------------------------------------------
# host-side sharding / assembly
# ----------------------------------------------------------------------------

def _to_bf16(a):
    import ml_dtypes
    return a.astype(ml_dtypes.bfloat16)


def make_in_maps(cfg: Cfg, student, teacher, center, targets):
    """Per-core input dicts. student/teacher/center are np.float32 full arrays."""
    D, BL, P, G = cfg.D, cfg.BL, cfg.P, cfg.G
    st3 = student.reshape(cfg.NCROPS, cfg.B, D)
    te3 = teacher.reshape(2, cfg.B, D)
    counts = np.bincount(targets, minlength=cfg.NCENT) * 2
    scale_t = (0.1 / np.maximum(counts, 1)).astype(np.float32)

    in_maps = []
    for k in range(cfg.NCORES):
        bsl = slice(k * BL, (k + 1) * BL)
        # transposed d-chunked layouts; col index within group g: c*cols + row
        S = st3[:, bsl, :].reshape(cfg.NCROPS * BL, D)   # rows v*BL+j
        S_t = np.ascontiguousarray(
            S.T.reshape(cfg.NGRP, G, P, cfg.SC).transpose(0, 2, 1, 3)
            .reshape(cfg.NGRP, P, G * cfg.SC))
        if cfg.bf16_student:
            S_t = _to_bf16(S_t)
        T = te3[:, bsl, :].reshape(2 * BL, D)            # rows i*BL+j
        SGG = cfg.SG * G
        T_t = np.ascontiguousarray(
            T.T.reshape(cfg.NSG, SGG, P, cfg.TC).transpose(0, 2, 1, 3)
            .reshape(cfg.NSG, P, SGG * cfg.TC))
        C = center[targets[bsl]]                          # [BL, D]
        C_t = np.ascontiguousarray(
            C.T.reshape(cfg.NSG, SGG, P, BL).transpose(0, 2, 1, 3)
            .reshape(cfg.NSG, P, SGG * BL))
        # teacher natural, packed into half-slices on the partition axis
        T_n = np.empty((cfg.NCORES, 2 * cfg.TC, cfg.DSH), np.float32)
        for s in range(cfg.NCORES):
            T_n[s, :cfg.TC] = T[:, s * cfg.DSL: s * cfg.DSL + cfg.DSH]
            T_n[s, cfg.TC:] = T[:, s * cfg.DSL + cfg.DSH:(s + 1) * cfg.DSL]
        # scaled one-hot, duplicated across both partition halves
        ohm = np.zeros((2 * cfg.TC, cfg.NCENT), np.float32)
        for i in range(2):
            for j in range(BL):
                t = targets[k * BL + j]
                ohm[i * BL + j, t] = scale_t[t]
        ohm[cfg.TC:] = ohm[:cfg.TC]
        # 0.9 * center for this core's d-slice, packed the same way
        c09 = np.empty((2 * cfg.NCENT, cfg.DSH), np.float32)
        c09[:cfg.NCENT] = 0.9 * center[:, k * cfg.DSL: k * cfg.DSL + cfg.DSH]
        c09[cfg.NCENT:] = 0.9 * center[:, k * cfg.DSL + cfg.DSH:(k + 1) * cfg.DSL]
        in_maps.append({
            "s_t": S_t, "t_t": T_t, "c_t": C_t, "t_n": T_n, "oh": ohm, "c09": c09,
        })
    return in_maps


def assemble(cfg: Cfg, results, center, temp, use_collective=True):
    BL = cfg.BL
    # ---- loss, assembled in float64 ----
    total = 0.0
    mask = np.ones((2, cfg.NCROPS))
    mask[0, 0] = 0.0
    mask[1, 1] = 0.0
    for k in range(cfg.NCORES):
        D4 = results[k]["dots_o"].astype(np.float64).reshape(2, BL, cfg.NCROPS, BL)
        d = D4[:, np.arange(BL), :, np.arange(BL)]        # [BL, 2, NCROPS]
        zs = results[k]["zs_o"].astype(np.float64).reshape(cfg.NCROPS, BL)
        ztp = results[k]["ztp_o"].astype(np.float64).reshape(cfg.G, 2, BL)
        zt = ztp.sum(axis=0)                              # [2, BL]
        term = (10.0 * d / zt.T[:, :, None]
                - np.log(zs).T[:, None, :])               # [BL, 2, NCROPS]
        total += (term * mask[None, :, :]).sum()
    loss = -total / ((2 * cfg.NCROPS - 2) * cfg.B)

    # ---- new_center ----
    full = np.empty((cfg.NCENT, cfg.D), np.float32)
    if use_collective:
        for k in range(cfg.NCORES):
            nco = results[k]["nco_o"]
            full[:, k * cfg.DSL: k * cfg.DSL + cfg.DSH] = nco[:cfg.NCENT]
            full[:, k * cfg.DSL + cfg.DSH:(k + 1) * cfg.DSL] = nco[cfg.NCENT:]
    else:
        bs = sum(results[k]["bcp_o"].astype(np.float64) for k in range(cfg.NCORES))
        for s in range(cfg.NCORES):
            blk = bs[s * 2 * cfg.NCENT:(s + 1) * 2 * cfg.NCENT]
            full[:, s * cfg.DSL: s * cfg.DSL + cfg.DSH] = blk[:cfg.NCENT]
            full[:, s * cfg.DSL + cfg.DSH:(s + 1) * cfg.DSL] = blk[cfg.NCENT:]
        full += np.float32(0.9) * center
    return np.float32(loss), full


_BUILT = {}


def _np(x, dtype=None):
    try:
        a = np.asarray(x)
    except Exception:
        import jax
        a = np.asarray(jax.device_get(x))
    return a.astype(dtype, copy=False) if dtype is not None else a


def kernel(student_output, teacher_output, center, targets, epoch, _want_trace=False):
    bass, bacc, mybir, tile, bass_utils = _import_concourse()
    student = _np(student_output, np.float32)
    teacher = _np(teacher_output, np.float32)
    center_np = _np(center, np.float32)
    targets_np = _np(targets).astype(np.int64)
    temp = _teacher_temp(int(_np(epoch)))

    cfg = Cfg(bf16_student=bool(int(os.environ.get("DINO_BF16", "1"))))
    use_cc = bool(int(os.environ.get("DINO_COLLECTIVE", "1")))
    key = (temp, cfg.bf16_student, use_cc)
    if key not in _BUILT:
        _BUILT[key] = build_kernel(cfg, temp, use_collective=use_cc)
    nc = _BUILT[key]

    in_maps = make_in_maps(cfg, student, teacher, center_np, targets_np)
    kw = {}
    if _want_trace:
        kw = dict(trace=True, trace_cores=list(range(cfg.NCORES)))
    res = bass_utils.run_bass_kernel_spmd(
        nc, in_maps, core_ids=list(range(cfg.NCORES)), **kw)
    out = assemble(cfg, res.results, center_np, temp, use_collective=use_cc)
    if _want_trace:
        return out, res
    return out


# revision 30
# speedup vs baseline: 1.8985x; 1.0615x over previous
"""Trainium2 Bass kernel for DINO loss (nn_DINOLoss_44083544326419).

kernel(**inputs) takes FULL unsharded inputs and returns (total_loss, new_center),
matching the reference nn.Module. Strategy:

- batch dim (B=256) sharded across 8 NeuronCores (32 samples/core)
- per core, heavy math runs in a d-chunked transposed layout
  ([128 d-partitions x row-columns]); full-row reductions become PE matmuls
  against a ones-vector, accumulated in PSUM across all 512 d-chunks:
    * dots[(i,b),(v,b')] = sum_d exp((t-c)/temp - K) * student
    * zs = sum_d exp(10*student)   (student softmax denominator)
    * ztp = sum_d qt               (teacher softmax denominator, per chunk-phase)
  student log-softmax needs no max-subtraction (|10 s|max ~ 55 fits f32 exp);
  teacher softmax uses a fixed offset K=100 (max u ~ 110 -> exp(u-K) <= e^10).
- center update: scatter-add as a one-hot matmul (one-hot columns pre-scaled by
  0.1/max(2*counts,1) on host), summed across cores with an on-device
  ReduceScatter; each core adds 0.9*center for its d-slice and writes that
  slice of new_center.
- tiny per-(i,v,b) scalar assembly (logs, masking, means) happens on host in
  float64 from the per-core reduction outputs (~85 KB/core).
"""

import os
import sys

import numpy as np

KOFF = 100.0  # teacher exp offset
WARMUP_T = 0.04
TEACHER_T = 0.07
WARMUP_EPOCHS = 30
NEPOCHS = 100


def _teacher_temp(epoch: int) -> float:
    sched = np.concatenate(
        (np.linspace(WARMUP_T, TEACHER_T, WARMUP_EPOCHS),
         np.ones(NEPOCHS - WARMUP_EPOCHS) * TEACHER_T))
    return float(sched[int(epoch)])


def _import_concourse():
    try:
        import concourse.bass  # noqa: F401
    except ImportError:
        for p in ("/opt/trn_rl_repo", "/root/.axon_site/_ro/trn_rl_repo"):
            if os.path.isdir(p) and p not in sys.path:
                sys.path.insert(0, p)
    import concourse.bass as bass
    import concourse.bacc as bacc
    import concourse.mybir as mybir
    import concourse.tile as tile
    from concourse import bass_utils
    return bass, bacc, mybir, tile, bass_utils


class Cfg:
    """Problem geometry. Defaults = the real problem; overridable for sim tests."""

    def __init__(self, D=65536, BL=32, NCROPS=10, NCENT=51, NCORES=8, G=16, SG=2,
                 bf16_student=True):
        self.D = D                    # feature dim
        self.BL = BL                  # local batch per core
        self.NCROPS = NCROPS
        self.NCENT = NCENT
        self.NCORES = NCORES
        self.B = BL * NCORES          # global batch
        self.P = 128                  # d-chunk partition size
        assert D % (self.P * NCORES * 2) == 0
        self.NCHUNK = D // self.P     # number of d-chunks
        self.G = G                    # chunks per student group
        assert self.NCHUNK % G == 0
        self.NGRP = self.NCHUNK // G
        self.SG = SG                  # student groups per teacher supergroup
        assert self.NGRP % SG == 0
        self.NSG = self.NGRP // SG
        self.TC = 2 * BL              # teacher cols per chunk
        self.SC = NCROPS * BL         # student cols per chunk
        assert 2 * self.TC <= 128
        self.DSL = D // NCORES        # d-slice width per core (center output)
        self.DSH = self.DSL // 2      # packed half-slice width
        self.SCB = min(512, self.DSH)          # scatter matmul free-dim block
        assert self.DSH % self.SCB == 0
        self.QW = min(4096, self.DSH)          # scatter staging width
        assert self.DSH % self.QW == 0
        self.PSB = min(1024, self.QW)          # scatter psum tile width
        assert self.QW % self.PSB == 0 and self.PSB % self.SCB == 0
        self.CQW = min(2048, self.DSH)         # phase-C tile width
        assert self.DSH % self.CQW == 0
        self.ZW = min(512, self.G * self.TC)   # ztp matmul slice width
        assert (self.G * self.TC) % self.ZW == 0
        self.bf16_student = bf16_student


def build_kernel(cfg: Cfg, temp: float, use_collective=True):
    """Builds, schedules and compiles the SPMD kernel; returns the Bacc module."""
    bass, bacc, mybir, tile, bass_utils = _import_concourse()
    dt = mybir.dt
    f32 = dt.float32
    # matmul-feeding dtype: bf16, or float32r (f32 bits, 1 cyc/row on PE when
    # the moving dim >= 256; walrus requires producers typed f32r end-to-end)
    dt_s = dt.bfloat16 if cfg.bf16_student else dt.float32r
    dt_r = dt.float32r
    AF = mybir.ActivationFunctionType

    def mmview(ap):
        return ap

    nc = bacc.Bacc("TRN2", target_bir_lowering=False, debug=False,
                   num_devices=cfg.NCORES)

    P, G, SG, TC, SC, NCENT, BL = cfg.P, cfg.G, cfg.SG, cfg.TC, cfg.SC, cfg.NCENT, cfg.BL

    s_t = nc.dram_tensor("s_t", [cfg.NGRP, P, G * SC], dt_s, kind="ExternalInput")
    t_t = nc.dram_tensor("t_t", [cfg.NSG, P, SG * G * TC], f32, kind="ExternalInput")
    c_t = nc.dram_tensor("c_t", [cfg.NSG, P, SG * G * BL], f32, kind="ExternalInput")
    t_n = nc.dram_tensor("t_n", [cfg.NCORES, 2 * TC, cfg.DSH], dt_r,
                         kind="ExternalInput")
    oh = nc.dram_tensor("oh", [2 * TC, NCENT], dt_r, kind="ExternalInput")
    c09 = nc.dram_tensor("c09", [2 * NCENT, cfg.DSH], f32, kind="ExternalInput")

    dots_o = nc.dram_tensor("dots_o", [TC, SC], f32, kind="ExternalOutput")
    zs_o = nc.dram_tensor("zs_o", [1, SC], f32, kind="ExternalOutput")
    ztp_o = nc.dram_tensor("ztp_o", [1, G * TC], f32, kind="ExternalOutput")
    if use_collective:
        nco_o = nc.dram_tensor("nco_o", [2 * NCENT, cfg.DSH], f32,
                               kind="ExternalOutput")
    else:
        bcp_o = nc.dram_tensor("bcp_o", [cfg.NCORES * 2 * NCENT, cfg.DSH], f32,
                               kind="ExternalOutput")

    with tile.TileContext(nc) as tc:
        with (
            tc.tile_pool(name="const", bufs=1) as constp,
            tc.tile_pool(name="tn", bufs=2) as tnp,
            tc.tile_pool(name="stag", bufs=2) as stagp,
            tc.tile_pool(name="spool", bufs=2) as spool,
            tc.tile_pool(name="espool", bufs=2) as espool,
            tc.tile_pool(name="tpool", bufs=2) as tpool,
            tc.tile_pool(name="cpool", bufs=2) as cpool,
            tc.tile_pool(name="qtpool", bufs=2) as qtpool,
            tc.tile_pool(name="opool", bufs=1) as opool,
            tc.tile_pool(name="cph", bufs=1) as cph,
            tc.tile_pool(name="psA", bufs=1, space="PSUM") as psA,
            tc.tile_pool(name="psS", bufs=2, space="PSUM") as psS,
            tc.tile_pool(name="dram", bufs=1, space="DRAM") as dram,
        ):
            ones_t = constp.tile([P, 1], dt_s, name="ones_t")
            nc.gpsimd.memset(ones_t[:], 1.0)
            koff_t = constp.tile([P, 1], f32, name="koff_t")
            nc.gpsimd.memset(koff_t[:], -KOFF)
            oh_t = constp.tile([2 * TC, NCENT], dt_r, name="oh_t")
            nc.gpsimd.dma_start(out=oh_t[:], in_=oh[:])

            if use_collective:
                cc_in = dram.tile([cfg.NCORES * 2 * NCENT, cfg.DSH], f32,
                                  name="cc_in")
                cc_out = dram.tile([2 * NCENT, cfg.DSH], f32, name="cc_out")

            # ---------- phase A slice emitter: scatter-add via one-hot matmul ----
            def phase_a_slice(s):
                tn_t = tnp.tile([2 * TC, cfg.DSH], dt_r, name="tn_t")
                nc.gpsimd.dma_start(out=tn_t[:], in_=t_n[s])
                for half in range(2):
                    pr0 = half * TC
                    row0 = s * 2 * NCENT + half * NCENT
                    for q in range(cfg.DSH // cfg.QW):
                        stag = stagp.tile([NCENT, cfg.QW], f32, name="stag")
                        for pb in range(cfg.QW // cfg.PSB):
                            bc_ps = psS.tile([NCENT, cfg.PSB], f32, name="bc_ps")
                            for m in range(cfg.PSB // cfg.SCB):
                                col = q * cfg.QW + pb * cfg.PSB + m * cfg.SCB
                                nc.tensor.matmul(
                                    bc_ps[:, m * cfg.SCB:(m + 1) * cfg.SCB],
                                    mmview(oh_t[pr0:pr0 + TC, :]),
                                    mmview(tn_t[pr0:pr0 + TC, col:col + cfg.SCB]),
                                    start=True, stop=True)
                            nc.vector.tensor_copy(
                                stag[:, pb * cfg.PSB:(pb + 1) * cfg.PSB], bc_ps[:])
                        dst = cc_in if use_collective else bcp_o
                        nc.gpsimd.dma_start(
                            out=dst[row0:row0 + NCENT,
                                    q * cfg.QW:(q + 1) * cfg.QW],
                            in_=stag[:])

            # ---------- phase B (with phase A slices interleaved) ----------
            dots_ps = psA.tile([TC, SC], f32, name="dots_ps")
            zs_ps = psA.tile([1, SC], f32, name="zs_ps")
            ztp_ps = psA.tile([1, G * TC], f32, name="ztp_ps")

            spsg = (cfg.NCORES + cfg.NSG - 1) // cfg.NSG  # phase-A slices per sg

            for sg in range(cfg.NSG):
                tsg = tpool.tile([P, SG * G * TC], f32, name="tsg")
                csg = cpool.tile([P, SG * G * BL], f32, name="csg")
                nc.gpsimd.dma_start(out=tsg[:], in_=t_t[sg])
                nc.gpsimd.dma_start(out=csg[:], in_=c_t[sg])
                # u = t - c (in place into tsg), for both teacher views i=0,1
                tv = tsg[:].rearrange("p (k c) -> p k c", c=TC)
                cv = csg[:].rearrange("p (k j) -> p k j", j=BL)
                nc.vector.tensor_sub(tv[:, :, 0:BL], tv[:, :, 0:BL], cv)
                nc.vector.tensor_sub(tv[:, :, BL:TC], tv[:, :, BL:TC], cv)
                # qt = exp(u/temp - K)
                qt = qtpool.tile([P, SG * G * TC], dt_s, name="qt")
                nc.scalar.activation(qt[:], tsg[:], AF.Exp,
                                     bias=koff_t[:], scale=float(1.0 / temp))

                for gl in range(SG):
                    g = sg * SG + gl
                    s_tile = spool.tile([P, G * SC], dt_s, name="s_tile")
                    # issue on the scalar engine's HWDGE queue: parallel
                    # descriptor generation with the gpsimd-queue loads
                    nc.scalar.dma_start(out=s_tile[:], in_=s_t[g])
                    es = espool.tile([P, G * SC], dt_s, name="es")
                    nc.scalar.activation(es[:], s_tile[:], AF.Exp, scale=10.0)
                    first = g == 0
                    last = g == cfg.NGRP - 1
                    for c in range(G):
                        qsl = qt[:, (gl * G + c) * TC:(gl * G + c + 1) * TC]
                        nc.tensor.matmul(
                            dots_ps[:], mmview(qsl),
                            mmview(s_tile[:, c * SC:(c + 1) * SC]),
                            start=first and c == 0, stop=last and c == G - 1)
                        nc.tensor.matmul(
                            zs_ps[:], mmview(ones_t[:]),
                            mmview(es[:, c * SC:(c + 1) * SC]),
                            start=first and c == 0, stop=last and c == G - 1)
                    for h in range(G * TC // cfg.ZW):
                        nc.tensor.matmul(
                            ztp_ps[:, h * cfg.ZW:(h + 1) * cfg.ZW],
                            mmview(ones_t[:]),
                            mmview(qt[:, gl * G * TC + h * cfg.ZW:
                                       gl * G * TC + (h + 1) * cfg.ZW]),
                            start=first, stop=last)

                for si in range(sg * spsg, min((sg + 1) * spsg, cfg.NCORES)):
                    phase_a_slice(si)

            if use_collective:
                nc.gpsimd.collective_compute(
                    "ReduceScatter",
                    mybir.AluOpType.add,
                    replica_groups=[list(range(cfg.NCORES))],
                    ins=[cc_in[:].opt()],
                    outs=[cc_out[:].opt()],
                )

            wout = SC + SC + G * TC
            outst = opool.tile([TC, wout], f32, name="outst")
            nc.scalar.copy(outst[:TC, :SC], dots_ps[:])
            nc.scalar.copy(outst[:1, SC:2 * SC], zs_ps[:])
            nc.scalar.copy(outst[:1, 2 * SC:wout], ztp_ps[:])
            nc.gpsimd.dma_start(out=dots_o[:], in_=outst[:TC, :SC])
            nc.gpsimd.dma_start(out=zs_o[:], in_=outst[:1, SC:2 * SC])
            nc.gpsimd.dma_start(out=ztp_o[:], in_=outst[:1, 2 * SC:wout])

            # ---------- phase C: += 0.9*center slice, emit new_center slice ----------
            if use_collective:
                for half in range(2):
                    for q in range(cfg.DSH // cfg.CQW):
                        cs = slice(q * cfg.CQW, (q + 1) * cfg.CQW)
                        r0 = half * NCENT
                        cc_t = cph.tile([NCENT, cfg.CQW], f32, name="cc_t")
                        c09_t = cph.tile([NCENT, cfg.CQW], f32, name="c09_t")
                        nc.gpsimd.dma_start(out=cc_t[:], in_=cc_out[r0:r0 + NCENT, cs])
                        nc.gpsimd.dma_start(out=c09_t[:], in_=c09[r0:r0 + NCENT, cs])
                        nc.vector.tensor_add(cc_t[:], cc_t[:], c09_t[:])
                        nc.gpsimd.dma_start(out=nco_o[r0:r0 + NCENT, cs], in_=cc_t[:])

    nc.compile()
    return nc


# ----------------------------------------------------------------------------
# host-side sharding / assembly
# ----------------------------------------------------------------------------

def _to_bf16(a):
    import ml_dtypes
    return a.astype(ml_dtypes.bfloat16)


def make_in_maps(cfg: Cfg, student, teacher, center, targets):
    """Per-core input dicts. student/teacher/center are np.float32 full arrays."""
    D, BL, P, G = cfg.D, cfg.BL, cfg.P, cfg.G
    st3 = student.reshape(cfg.NCROPS, cfg.B, D)
    te3 = teacher.reshape(2, cfg.B, D)
    counts = np.bincount(targets, minlength=cfg.NCENT) * 2
    scale_t = (0.1 / np.maximum(counts, 1)).astype(np.float32)

    in_maps = []
    for k in range(cfg.NCORES):
        bsl = slice(k * BL, (k + 1) * BL)
        # transposed d-chunked layouts; col index within group g: c*cols + row
        S = st3[:, bsl, :].reshape(cfg.NCROPS * BL, D)   # rows v*BL+j
        S_t = np.ascontiguousarray(
            S.T.reshape(cfg.NGRP, G, P, cfg.SC).transpose(0, 2, 1, 3)
            .reshape(cfg.NGRP, P, G * cfg.SC))
        if cfg.bf16_student:
            S_t = _to_bf16(S_t)
        T = te3[:, bsl, :].reshape(2 * BL, D)            # rows i*BL+j
        SGG = cfg.SG * G
        T_t = np.ascontiguousarray(
            T.T.reshape(cfg.NSG, SGG, P, cfg.TC).transpose(0, 2, 1, 3)
            .reshape(cfg.NSG, P, SGG * cfg.TC))
        C = center[targets[bsl]]                          # [BL, D]
        C_t = np.ascontiguousarray(
            C.T.reshape(cfg.NSG, SGG, P, BL).transpose(0, 2, 1, 3)
            .reshape(cfg.NSG, P, SGG * BL))
        # teacher natural, packed into half-slices on the partition axis
        T_n = np.empty((cfg.NCORES, 2 * cfg.TC, cfg.DSH), np.float32)
        for s in range(cfg.NCORES):
            T_n[s, :cfg.TC] = T[:, s * cfg.DSL: s * cfg.DSL + cfg.DSH]
            T_n[s, cfg.TC:] = T[:, s * cfg.DSL + cfg.DSH:(s + 1) * cfg.DSL]
        # scaled one-hot, duplicated across both partition halves
        ohm = np.zeros((2 * cfg.TC, cfg.NCENT), np.float32)
        for i in range(2):
            for j in range(BL):
                t = targets[k * BL + j]
                ohm[i * BL + j, t] = scale_t[t]
        ohm[cfg.TC:] = ohm[:cfg.TC]
        # 0.9 * center for this core's d-slice, packed the same way
        c09 = np.empty((2 * cfg.NCENT, cfg.DSH), np.float32)
        c09[:cfg.NCENT] = 0.9 * center[:, k * cfg.DSL: k * cfg.DSL + cfg.DSH]
        c09[cfg.NCENT:] = 0.9 * center[:, k * cfg.DSL + cfg.DSH:(k + 1) * cfg.DSL]
        in_maps.append({
            "s_t": S_t, "t_t": T_t, "c_t": C_t, "t_n": T_n, "oh": ohm, "c09": c09,
        })
    return in_maps


def assemble(cfg: Cfg, results, center, temp, use_collective=True):
    BL = cfg.BL
    # ---- loss, assembled in float64 ----
    total = 0.0
    mask = np.ones((2, cfg.NCROPS))
    mask[0, 0] = 0.0
    mask[1, 1] = 0.0
    for k in range(cfg.NCORES):
        D4 = results[k]["dots_o"].astype(np.float64).reshape(2, BL, cfg.NCROPS, BL)
        d = D4[:, np.arange(BL), :, np.arange(BL)]        # [BL, 2, NCROPS]
        zs = results[k]["zs_o"].astype(np.float64).reshape(cfg.NCROPS, BL)
        ztp = results[k]["ztp_o"].astype(np.float64).reshape(cfg.G, 2, BL)
        zt = ztp.sum(axis=0)                              # [2, BL]
        term = (10.0 * d / zt.T[:, :, None]
                - np.log(zs).T[:, None, :])               # [BL, 2, NCROPS]
        total += (term * mask[None, :, :]).sum()
    loss = -total / ((2 * cfg.NCROPS - 2) * cfg.B)

    # ---- new_center ----
    full = np.empty((cfg.NCENT, cfg.D), np.float32)
    if use_collective:
        for k in range(cfg.NCORES):
            nco = results[k]["nco_o"]
            full[:, k * cfg.DSL: k * cfg.DSL + cfg.DSH] = nco[:cfg.NCENT]
            full[:, k * cfg.DSL + cfg.DSH:(k + 1) * cfg.DSL] = nco[cfg.NCENT:]
    else:
        bs = sum(results[k]["bcp_o"].astype(np.float64) for k in range(cfg.NCORES))
        for s in range(cfg.NCORES):
            blk = bs[s * 2 * cfg.NCENT:(s + 1) * 2 * cfg.NCENT]
            full[:, s * cfg.DSL: s * cfg.DSL + cfg.DSH] = blk[:cfg.NCENT]
            full[:, s * cfg.DSL + cfg.DSH:(s + 1) * cfg.DSL] = blk[cfg.NCENT:]
        full += np.float32(0.9) * center
    return np.float32(loss), full


_BUILT = {}


def _np(x, dtype=None):
    try:
        a = np.asarray(x)
    except Exception:
        import jax
        a = np.asarray(jax.device_get(x))
    return a.astype(dtype, copy=False) if dtype is not None else a


def kernel(student_output, teacher_output, center, targets, epoch, _want_trace=False):
    bass, bacc, mybir, tile, bass_utils = _import_concourse()
    student = _np(student_output, np.float32)
    teacher = _np(teacher_output, np.float32)
    center_np = _np(center, np.float32)
    targets_np = _np(targets).astype(np.int64)
    temp = _teacher_temp(int(_np(epoch)))

    cfg = Cfg(bf16_student=bool(int(os.environ.get("DINO_BF16", "1"))))
    # Default: device computes the scatter-add partials; the 8-way cross-core
    # sum + 0.9*center happen on host during unshard (measured 421 us vs
    # 602-752 us with the on-device ReduceScatter, whose DMA-suppression
    # window dominates). Set DINO_COLLECTIVE=1 for the on-device all-reduce.
    use_cc = bool(int(os.environ.get("DINO_COLLECTIVE", "0")))
    key = (temp, cfg.bf16_student, use_cc)
    if key not in _BUILT:
        _BUILT[key] = build_kernel(cfg, temp, use_collective=use_cc)
    nc = _BUILT[key]

    in_maps = make_in_maps(cfg, student, teacher, center_np, targets_np)
    kw = {}
    if _want_trace:
        kw = dict(trace=True, trace_cores=list(range(cfg.NCORES)))
    res = bass_utils.run_bass_kernel_spmd(
        nc, in_maps, core_ids=list(range(cfg.NCORES)), **kw)
    out = assemble(cfg, res.results, center_np, temp, use_collective=use_cc)
    if _want_trace:
        return out, res
    return out


# revision 31
# speedup vs baseline: 1.9320x; 1.0176x over previous
"""Trainium2 Bass kernel for DINO loss (nn_DINOLoss_44083544326419).

kernel(**inputs) takes FULL unsharded inputs and returns (total_loss, new_center),
matching the reference nn.Module. Strategy:

- batch dim (B=256) sharded across 8 NeuronCores (32 samples/core)
- per core, heavy math runs in a d-chunked transposed layout
  ([128 d-partitions x row-columns]); full-row reductions become PE matmuls
  against a ones-vector, accumulated in PSUM across all 512 d-chunks:
    * dots[(i,b),(v,b')] = sum_d exp((t-c)/temp - K) * student
    * zs = sum_d exp(10*student)   (student softmax denominator)
    * ztp = sum_d qt               (teacher softmax denominator, per chunk-phase)
  student log-softmax needs no max-subtraction (|10 s|max ~ 55 fits f32 exp);
  teacher softmax uses a fixed offset K=100 (max u ~ 110 -> exp(u-K) <= e^10).
- center update: scatter-add as a one-hot matmul (one-hot columns pre-scaled by
  0.1/max(2*counts,1) on host), summed across cores with an on-device
  ReduceScatter; each core adds 0.9*center for its d-slice and writes that
  slice of new_center.
- tiny per-(i,v,b) scalar assembly (logs, masking, means) happens on host in
  float64 from the per-core reduction outputs (~85 KB/core).
"""

import os
import sys

import numpy as np

KOFF = 100.0  # teacher exp offset
WARMUP_T = 0.04
TEACHER_T = 0.07
WARMUP_EPOCHS = 30
NEPOCHS = 100


def _teacher_temp(epoch: int) -> float:
    sched = np.concatenate(
        (np.linspace(WARMUP_T, TEACHER_T, WARMUP_EPOCHS),
         np.ones(NEPOCHS - WARMUP_EPOCHS) * TEACHER_T))
    return float(sched[int(epoch)])


def _import_concourse():
    try:
        import concourse.bass  # noqa: F401
    except ImportError:
        for p in ("/opt/trn_rl_repo", "/root/.axon_site/_ro/trn_rl_repo"):
            if os.path.isdir(p) and p not in sys.path:
                sys.path.insert(0, p)
    import concourse.bass as bass
    import concourse.bacc as bacc
    import concourse.mybir as mybir
    import concourse.tile as tile
    from concourse import bass_utils
    return bass, bacc, mybir, tile, bass_utils


class Cfg:
    """Problem geometry. Defaults = the real problem; overridable for sim tests."""

    def __init__(self, D=65536, BL=32, NCROPS=10, NCENT=51, NCORES=8, G=16, SG=2,
                 bf16_student=True):
        self.D = D                    # feature dim
        self.BL = BL                  # local batch per core
        self.NCROPS = NCROPS
        self.NCENT = NCENT
        self.NCORES = NCORES
        self.B = BL * NCORES          # global batch
        self.P = 128                  # d-chunk partition size
        assert D % (self.P * NCORES * 2) == 0
        self.NCHUNK = D // self.P     # number of d-chunks
        self.G = G                    # chunks per student group
        assert self.NCHUNK % G == 0
        self.NGRP = self.NCHUNK // G
        self.SG = SG                  # student groups per teacher supergroup
        assert self.NGRP % SG == 0
        self.NSG = self.NGRP // SG
        self.TC = 2 * BL              # teacher cols per chunk
        self.SC = NCROPS * BL         # student cols per chunk
        assert 2 * self.TC <= 128
        self.DSL = D // NCORES        # d-slice width per core (center output)
        self.DSH = self.DSL // 2      # packed half-slice width
        self.SCB = min(512, self.DSH)          # scatter matmul free-dim block
        assert self.DSH % self.SCB == 0
        self.QW = min(4096, self.DSH)          # scatter staging width
        assert self.DSH % self.QW == 0
        self.PSB = min(1024, self.QW)          # scatter psum tile width
        assert self.QW % self.PSB == 0 and self.PSB % self.SCB == 0
        self.CQW = min(2048, self.DSH)         # phase-C tile width
        assert self.DSH % self.CQW == 0
        self.ZW = min(512, self.G * self.TC)   # ztp matmul slice width
        assert (self.G * self.TC) % self.ZW == 0
        self.bf16_student = bf16_student


def build_kernel(cfg: Cfg, temp: float, use_collective=True):
    """Builds, schedules and compiles the SPMD kernel; returns the Bacc module."""
    bass, bacc, mybir, tile, bass_utils = _import_concourse()
    dt = mybir.dt
    f32 = dt.float32
    # matmul-feeding dtype: bf16, or float32r (f32 bits, 1 cyc/row on PE when
    # the moving dim >= 256; walrus requires producers typed f32r end-to-end)
    dt_s = dt.bfloat16 if cfg.bf16_student else dt.float32r
    dt_r = dt.float32r
    AF = mybir.ActivationFunctionType

    def mmview(ap):
        return ap

    nc = bacc.Bacc("TRN2", target_bir_lowering=False, debug=False,
                   num_devices=cfg.NCORES)

    P, G, SG, TC, SC, NCENT, BL = cfg.P, cfg.G, cfg.SG, cfg.TC, cfg.SC, cfg.NCENT, cfg.BL

    s_t = nc.dram_tensor("s_t", [cfg.NGRP, P, G * SC], dt_s, kind="ExternalInput")
    t_t = nc.dram_tensor("t_t", [cfg.NSG, P, SG * G * TC], f32, kind="ExternalInput")
    c_t = nc.dram_tensor("c_t", [cfg.NSG, P, SG * G * BL], f32, kind="ExternalInput")
    t_n = nc.dram_tensor("t_n", [cfg.NCORES, 2 * TC, cfg.DSH], dt_r,
                         kind="ExternalInput")
    oh = nc.dram_tensor("oh", [2 * TC, NCENT], dt_r, kind="ExternalInput")
    c09 = nc.dram_tensor("c09", [2 * NCENT, cfg.DSH], f32, kind="ExternalInput")

    dots_o = nc.dram_tensor("dots_o", [TC, SC], f32, kind="ExternalOutput")
    zs_o = nc.dram_tensor("zs_o", [1, SC], f32, kind="ExternalOutput")
    ztp_o = nc.dram_tensor("ztp_o", [1, G * TC], f32, kind="ExternalOutput")
    if use_collective:
        nco_o = nc.dram_tensor("nco_o", [2 * NCENT, cfg.DSH], f32,
                               kind="ExternalOutput")
    else:
        bcp_o = nc.dram_tensor("bcp_o", [cfg.NCORES * 2 * NCENT, cfg.DSH], f32,
                               kind="ExternalOutput")

    with tile.TileContext(nc) as tc:
        with (
            tc.tile_pool(name="const", bufs=1) as constp,
            tc.tile_pool(name="tn", bufs=2) as tnp,
            tc.tile_pool(name="stag", bufs=2) as stagp,
            tc.tile_pool(name="spool", bufs=3) as spool,
            tc.tile_pool(name="espool", bufs=3) as espool,
            tc.tile_pool(name="tpool", bufs=2) as tpool,
            tc.tile_pool(name="cpool", bufs=2) as cpool,
            tc.tile_pool(name="qtpool", bufs=2) as qtpool,
            tc.tile_pool(name="opool", bufs=1) as opool,
            tc.tile_pool(name="cph", bufs=1) as cph,
            tc.tile_pool(name="psA", bufs=1, space="PSUM") as psA,
            tc.tile_pool(name="psS", bufs=2, space="PSUM") as psS,
            tc.tile_pool(name="dram", bufs=1, space="DRAM") as dram,
        ):
            ones_t = constp.tile([P, 1], dt_s, name="ones_t")
            nc.gpsimd.memset(ones_t[:], 1.0)
            koff_t = constp.tile([P, 1], f32, name="koff_t")
            nc.gpsimd.memset(koff_t[:], -KOFF)
            oh_t = constp.tile([2 * TC, NCENT], dt_r, name="oh_t")
            nc.gpsimd.dma_start(out=oh_t[:], in_=oh[:])

            if use_collective:
                cc_in = dram.tile([cfg.NCORES * 2 * NCENT, cfg.DSH], f32,
                                  name="cc_in")
                cc_out = dram.tile([2 * NCENT, cfg.DSH], f32, name="cc_out")

            # ---------- phase A slice emitter: scatter-add via one-hot matmul ----
            def phase_a_slice(s):
                tn_t = tnp.tile([2 * TC, cfg.DSH], dt_r, name="tn_t")
                nc.gpsimd.dma_start(out=tn_t[:], in_=t_n[s])
                for half in range(2):
                    pr0 = half * TC
                    row0 = s * 2 * NCENT + half * NCENT
                    for q in range(cfg.DSH // cfg.QW):
                        stag = stagp.tile([NCENT, cfg.QW], f32, name="stag")
                        for pb in range(cfg.QW // cfg.PSB):
                            bc_ps = psS.tile([NCENT, cfg.PSB], f32, name="bc_ps")
                            for m in range(cfg.PSB // cfg.SCB):
                                col = q * cfg.QW + pb * cfg.PSB + m * cfg.SCB
                                nc.tensor.matmul(
                                    bc_ps[:, m * cfg.SCB:(m + 1) * cfg.SCB],
                                    mmview(oh_t[pr0:pr0 + TC, :]),
                                    mmview(tn_t[pr0:pr0 + TC, col:col + cfg.SCB]),
                                    start=True, stop=True)
                            nc.vector.tensor_copy(
                                stag[:, pb * cfg.PSB:(pb + 1) * cfg.PSB], bc_ps[:])
                        dst = cc_in if use_collective else bcp_o
                        nc.gpsimd.dma_start(
                            out=dst[row0:row0 + NCENT,
                                    q * cfg.QW:(q + 1) * cfg.QW],
                            in_=stag[:])

            # ---------- phase B (with phase A slices interleaved) ----------
            dots_ps = psA.tile([TC, SC], f32, name="dots_ps")
            zs_ps = psA.tile([1, SC], f32, name="zs_ps")
            ztp_ps = psA.tile([1, G * TC], f32, name="ztp_ps")

            spsg = (cfg.NCORES + cfg.NSG - 1) // cfg.NSG  # phase-A slices per sg

            for sg in range(cfg.NSG):
                tsg = tpool.tile([P, SG * G * TC], f32, name="tsg")
                csg = cpool.tile([P, SG * G * BL], f32, name="csg")
                teng = nc.sync if not use_collective else nc.gpsimd
                teng.dma_start(out=tsg[:], in_=t_t[sg])
                teng.dma_start(out=csg[:], in_=c_t[sg])
                # u = t - c (in place into tsg), for both teacher views i=0,1
                tv = tsg[:].rearrange("p (k c) -> p k c", c=TC)
                cv = csg[:].rearrange("p (k j) -> p k j", j=BL)
                nc.vector.tensor_sub(tv[:, :, 0:BL], tv[:, :, 0:BL], cv)
                nc.vector.tensor_sub(tv[:, :, BL:TC], tv[:, :, BL:TC], cv)
                # qt = exp(u/temp - K)
                qt = qtpool.tile([P, SG * G * TC], dt_s, name="qt")
                nc.scalar.activation(qt[:], tsg[:], AF.Exp,
                                     bias=koff_t[:], scale=float(1.0 / temp))

                for gl in range(SG):
                    g = sg * SG + gl
                    s_tile = spool.tile([P, G * SC], dt_s, name="s_tile")
                    # issue on the scalar engine's HWDGE queue: parallel
                    # descriptor generation with the gpsimd-queue loads
                    nc.scalar.dma_start(out=s_tile[:], in_=s_t[g])
                    es = espool.tile([P, G * SC], dt_s, name="es")
                    nc.scalar.activation(es[:], s_tile[:], AF.Exp, scale=10.0)
                    first = g == 0
                    last = g == cfg.NGRP - 1
                    for c in range(G):
                        qsl = qt[:, (gl * G + c) * TC:(gl * G + c + 1) * TC]
                        nc.tensor.matmul(
                            dots_ps[:], mmview(qsl),
                            mmview(s_tile[:, c * SC:(c + 1) * SC]),
                            start=first and c == 0, stop=last and c == G - 1)
                        nc.tensor.matmul(
                            zs_ps[:], mmview(ones_t[:]),
                            mmview(es[:, c * SC:(c + 1) * SC]),
                            start=first and c == 0, stop=last and c == G - 1)
                    for h in range(G * TC // cfg.ZW):
                        nc.tensor.matmul(
                            ztp_ps[:, h * cfg.ZW:(h + 1) * cfg.ZW],
                            mmview(ones_t[:]),
                            mmview(qt[:, gl * G * TC + h * cfg.ZW:
                                       gl * G * TC + (h + 1) * cfg.ZW]),
                            start=first, stop=last)

                for si in range(sg * spsg, min((sg + 1) * spsg, cfg.NCORES)):
                    phase_a_slice(si)

            if use_collective:
                nc.gpsimd.collective_compute(
                    "ReduceScatter",
                    mybir.AluOpType.add,
                    replica_groups=[list(range(cfg.NCORES))],
                    ins=[cc_in[:].opt()],
                    outs=[cc_out[:].opt()],
                )

            wout = SC + SC + G * TC
            outst = opool.tile([TC, wout], f32, name="outst")
            nc.scalar.copy(outst[:TC, :SC], dots_ps[:])
            nc.scalar.copy(outst[:1, SC:2 * SC], zs_ps[:])
            nc.scalar.copy(outst[:1, 2 * SC:wout], ztp_ps[:])
            nc.gpsimd.dma_start(out=dots_o[:], in_=outst[:TC, :SC])
            nc.gpsimd.dma_start(out=zs_o[:], in_=outst[:1, SC:2 * SC])
            nc.gpsimd.dma_start(out=ztp_o[:], in_=outst[:1, 2 * SC:wout])

            # ---------- phase C: += 0.9*center slice, emit new_center slice ----------
            if use_collective:
                for half in range(2):
                    for q in range(cfg.DSH // cfg.CQW):
                        cs = slice(q * cfg.CQW, (q + 1) * cfg.CQW)
                        r0 = half * NCENT
                        cc_t = cph.tile([NCENT, cfg.CQW], f32, name="cc_t")
                        c09_t = cph.tile([NCENT, cfg.CQW], f32, name="c09_t")
                        nc.gpsimd.dma_start(out=cc_t[:], in_=cc_out[r0:r0 + NCENT, cs])
                        nc.gpsimd.dma_start(out=c09_t[:], in_=c09[r0:r0 + NCENT, cs])
                        nc.vector.tensor_add(cc_t[:], cc_t[:], c09_t[:])
                        nc.gpsimd.dma_start(out=nco_o[r0:r0 + NCENT, cs], in_=cc_t[:])

    nc.compile()
    return nc


# ----------------------------------------------------------------------------
# host-side sharding / assembly
# ----------------------------------------------------------------------------

def _to_bf16(a):
    import ml_dtypes
    return a.astype(ml_dtypes.bfloat16)


def make_in_maps(cfg: Cfg, student, teacher, center, targets):
    """Per-core input dicts. student/teacher/center are np.float32 full arrays."""
    D, BL, P, G = cfg.D, cfg.BL, cfg.P, cfg.G
    st3 = student.reshape(cfg.NCROPS, cfg.B, D)
    te3 = teacher.reshape(2, cfg.B, D)
    counts = np.bincount(targets, minlength=cfg.NCENT) * 2
    scale_t = (0.1 / np.maximum(counts, 1)).astype(np.float32)

    in_maps = []
    for k in range(cfg.NCORES):
        bsl = slice(k * BL, (k + 1) * BL)
        # transposed d-chunked layouts; col index within group g: c*cols + row
        S = st3[:, bsl, :].reshape(cfg.NCROPS * BL, D)   # rows v*BL+j
        S_t = np.ascontiguousarray(
            S.T.reshape(cfg.NGRP, G, P, cfg.SC).transpose(0, 2, 1, 3)
            .reshape(cfg.NGRP, P, G * cfg.SC))
        if cfg.bf16_student:
            S_t = _to_bf16(S_t)
        T = te3[:, bsl, :].reshape(2 * BL, D)            # rows i*BL+j
        SGG = cfg.SG * G
        T_t = np.ascontiguousarray(
            T.T.reshape(cfg.NSG, SGG, P, cfg.TC).transpose(0, 2, 1, 3)
            .reshape(cfg.NSG, P, SGG * cfg.TC))
        C = center[targets[bsl]]                          # [BL, D]
        C_t = np.ascontiguousarray(
            C.T.reshape(cfg.NSG, SGG, P, BL).transpose(0, 2, 1, 3)
            .reshape(cfg.NSG, P, SGG * BL))
        # teacher natural, packed into half-slices on the partition axis
        T_n = np.empty((cfg.NCORES, 2 * cfg.TC, cfg.DSH), np.float32)
        for s in range(cfg.NCORES):
            T_n[s, :cfg.TC] = T[:, s * cfg.DSL: s * cfg.DSL + cfg.DSH]
            T_n[s, cfg.TC:] = T[:, s * cfg.DSL + cfg.DSH:(s + 1) * cfg.DSL]
        # scaled one-hot, duplicated across both partition halves
        ohm = np.zeros((2 * cfg.TC, cfg.NCENT), np.float32)
        for i in range(2):
            for j in range(BL):
                t = targets[k * BL + j]
                ohm[i * BL + j, t] = scale_t[t]
        ohm[cfg.TC:] = ohm[:cfg.TC]
        # 0.9 * center for this core's d-slice, packed the same way
        c09 = np.empty((2 * cfg.NCENT, cfg.DSH), np.float32)
        c09[:cfg.NCENT] = 0.9 * center[:, k * cfg.DSL: k * cfg.DSL + cfg.DSH]
        c09[cfg.NCENT:] = 0.9 * center[:, k * cfg.DSL + cfg.DSH:(k + 1) * cfg.DSL]
        in_maps.append({
            "s_t": S_t, "t_t": T_t, "c_t": C_t, "t_n": T_n, "oh": ohm, "c09": c09,
        })
    return in_maps


def assemble(cfg: Cfg, results, center, temp, use_collective=True):
    BL = cfg.BL
    # ---- loss, assembled in float64 ----
    total = 0.0
    mask = np.ones((2, cfg.NCROPS))
    mask[0, 0] = 0.0
    mask[1, 1] = 0.0
    for k in range(cfg.NCORES):
        D4 = results[k]["dots_o"].astype(np.float64).reshape(2, BL, cfg.NCROPS, BL)
        d = D4[:, np.arange(BL), :, np.arange(BL)]        # [BL, 2, NCROPS]
        zs = results[k]["zs_o"].astype(np.float64).reshape(cfg.NCROPS, BL)
        ztp = results[k]["ztp_o"].astype(np.float64).reshape(cfg.G, 2, BL)
        zt = ztp.sum(axis=0)                              # [2, BL]
        term = (10.0 * d / zt.T[:, :, None]
                - np.log(zs).T[:, None, :])               # [BL, 2, NCROPS]
        total += (term * mask[None, :, :]).sum()
    loss = -total / ((2 * cfg.NCROPS - 2) * cfg.B)

    # ---- new_center ----
    full = np.empty((cfg.NCENT, cfg.D), np.float32)
    if use_collective:
        for k in range(cfg.NCORES):
            nco = results[k]["nco_o"]
            full[:, k * cfg.DSL: k * cfg.DSL + cfg.DSH] = nco[:cfg.NCENT]
            full[:, k * cfg.DSL + cfg.DSH:(k + 1) * cfg.DSL] = nco[cfg.NCENT:]
    else:
        bs = sum(results[k]["bcp_o"].astype(np.float64) for k in range(cfg.NCORES))
        for s in range(cfg.NCORES):
            blk = bs[s * 2 * cfg.NCENT:(s + 1) * 2 * cfg.NCENT]
            full[:, s * cfg.DSL: s * cfg.DSL + cfg.DSH] = blk[:cfg.NCENT]
            full[:, s * cfg.DSL + cfg.DSH:(s + 1) * cfg.DSL] = blk[cfg.NCENT:]
        full += np.float32(0.9) * center
    return np.float32(loss), full


_BUILT = {}


def _np(x, dtype=None):
    try:
        a = np.asarray(x)
    except Exception:
        import jax
        a = np.asarray(jax.device_get(x))
    return a.astype(dtype, copy=False) if dtype is not None else a


def kernel(student_output, teacher_output, center, targets, epoch, _want_trace=False):
    bass, bacc, mybir, tile, bass_utils = _import_concourse()
    student = _np(student_output, np.float32)
    teacher = _np(teacher_output, np.float32)
    center_np = _np(center, np.float32)
    targets_np = _np(targets).astype(np.int64)
    temp = _teacher_temp(int(_np(epoch)))

    cfg = Cfg(bf16_student=bool(int(os.environ.get("DINO_BF16", "1"))))
    # Default: device computes the scatter-add partials; the 8-way cross-core
    # sum + 0.9*center happen on host during unshard (measured 421 us vs
    # 602-752 us with the on-device ReduceScatter, whose DMA-suppression
    # window dominates). Set DINO_COLLECTIVE=1 for the on-device all-reduce.
    use_cc = bool(int(os.environ.get("DINO_COLLECTIVE", "0")))
    key = (temp, cfg.bf16_student, use_cc)
    if key not in _BUILT:
        _BUILT[key] = build_kernel(cfg, temp, use_collective=use_cc)
    nc = _BUILT[key]

    in_maps = make_in_maps(cfg, student, teacher, center_np, targets_np)
    kw = {}
    if _want_trace:
        kw = dict(trace=True, trace_cores=list(range(cfg.NCORES)))
    res = bass_utils.run_bass_kernel_spmd(
        nc, in_maps, core_ids=list(range(cfg.NCORES)), **kw)
    out = assemble(cfg, res.results, center_np, temp, use_collective=use_cc)
    if _want_trace:
        return out, res
    return out
